# revision 20
# baseline (speedup 1.0000x reference)
"""GCN (3-layer + mean-pool + linear + softmax) on 8 Trainium2 NeuronCores.

Push-mode graph parallelism: each core owns a contiguous 12,500-node range
(padded 12544 = 128 lanes x 98 windows). Per layer, each core:
  phase A:  y~ = dinv * (x @ W) on PE, written as a bf16 table with duplicated
            rows [y~|y~] (256B rows) to DRAM,
  gathers   its OWN out-edge sources from the local table (int16 row ids,
            single chunk, large batched dma_gather ops),
  scatters  messages into per-global-window PSUM accumulators via fused
            one-hot matmuls: lhsT = (iota==dstlane)*|w| built in one
            tensor_scalar(is_equal,mult) op (DVE 4x mode / gpsimd),
  writes    bf16 partial sums [100352, 64] (partition-major rows -> large
            contiguous DMA descriptors),
  ReduceScatter (out 12544x64 bf16 ~= 55us) delivers summed aggregates for its
            own nodes; wide fused epilogue: x' = relu(dinv*(rs + y~self) + b).
Static SPMD schedule: superblocks of 7 windows with ~15 blocks each; blocks
serve a primary window and optionally the next (straddle), absorbing per-core
count variation with <=7% slot padding. Pooling via one-hot(graph) matmuls;
host applies the final 64x10 linear + softmax.
"""
import os
import sys
import numpy as np

sys.path.insert(0, os.path.dirname(os.path.abspath(__file__)))

N_NODES = 100000
N_GRAPHS = 256
IN_DIM = 128
F = 64
C = 8
NODES_C = 12500
PADN = 12544
W = 98
SBW = 7
NSB = W // SBW
GW = C * W
GSB = C * NSB
OPN = 8192
POOL_BUILD_FRAC = 0.15   # fraction of one-hot builds on gpsimd (Pool)

_prog_cache = {}


def _win_sched(B):
    return np.minimum((np.arange(B) * SBW) // B, SBW - 1)


# --------------------------------------------------------------------------
def _split_waits(nc, cap=1):
    """Walrus rejects >1 sem wait per instruction; hoist extras onto injected
    same-engine InstEventSemaphore waits."""
    import concourse.mybir as mybir
    uid = [0]
    n_fixed = 0
    for fn in nc.m.functions:
        for bb in fn.blocks:
            insts = bb.instructions
            new_list = []
            for inst in insts:
                si = inst.sync_info
                waits = list(si.on_wait) if si and si.on_wait else []
                if len(waits) > cap:
                    extra, keep = waits[:-cap], waits[-cap:]
                    for wv in extra:
                        uid[0] += 1
                        nop = mybir.InstEventSemaphore(name=f"waitfix_{uid[0]}")
                        nop.engine = inst.engine
                        nop.sync_info = mybir.SyncInfo(on_wait=[wv], on_update=[])
                        new_list.append(nop)
                    si.on_wait = keep
                    n_fixed += 1
                new_list.append(inst)
            if len(new_list) != len(insts):
                try:
                    bb.instructions = new_list
                except Exception:
                    insts.clear()
                    insts.extend(new_list)
    return n_fixed


# --------------------------------------------------------------------------
def _host_prep(x, edge_weight, edge_index, batch):
    src = np.asarray(edge_index[0], dtype=np.int64)
    dst = np.asarray(edge_index[1], dtype=np.int64)
    w_abs = np.abs(np.asarray(edge_weight, dtype=np.float32))
    batch = np.asarray(batch, dtype=np.int64)
    x = np.asarray(x, dtype=np.float32)

    deg = np.bincount(dst, weights=w_abs.astype(np.float64), minlength=N_NODES) + 1.0
    dinv_full = (1.0 / np.sqrt(deg)).astype(np.float32)

    ks = src // NODES_C
    so = src - ks * NODES_C
    srow = ((so % 128) * W + so // 128).astype(np.int16)
    kd = dst // NODES_C
    do = dst - kd * NODES_C
    lane = (do % 128).astype(np.float32)
    g = kd * W + do // 128

    gsb = g // SBW
    key = (ks * GSB + gsb) * SBW + (g % SBW)
    order = np.argsort(key, kind="stable")
    ks_s, g_s = ks[order], g[order]
    srow_s, lane_s, w_s = srow[order], lane[order], w_abs[order]

    cnt_cw = np.zeros((C, GW), dtype=np.int64)
    np.add.at(cnt_cw, (ks_s, g_s), 1)
    cnt_csb = cnt_cw.reshape(C, GSB, SBW).sum(axis=2)

    B_sb = np.maximum(np.ceil(cnt_csb.max(axis=0) / 128).astype(np.int64), SBW)

    def feasible(sb, B):
        w1 = _win_sched(B)
        firsts = [np.where((w1 == v) | (w1 == v - 1))[0][0] for v in range(SBW)]
        lasts = [np.where(w1 == v)[0][-1] for v in range(SBW)]
        for c in range(C):
            pos = 0
            for v in range(SBW):
                pos = max(pos, int(firsts[v]) * 128)
                pos += cnt_cw[c, sb * SBW + v]
                if pos > (int(lasts[v]) + 1) * 128:
                    return False
        return True

    for sb in range(GSB):
        while not feasible(sb, int(B_sb[sb])):
            B_sb[sb] += 1

    blk_of_sb = np.concatenate([[0], np.cumsum(B_sb)])
    NBLK = int(blk_of_sb[-1])
    TOT = NBLK * 128

    flat_cnt = np.zeros(C * GW, dtype=np.int64)
    np.add.at(flat_cnt, ks_s * GW + g_s, 1)
    flat_start = np.concatenate([[0], np.cumsum(flat_cnt)])

    idx_slot = np.zeros((C, TOT), dtype=np.int16)
    w_slot = np.zeros((C, TOT), dtype=np.float32)
    win_slot = np.full((C, TOT), -1, dtype=np.int64)
    lane_slot = np.full((C, TOT), -1.0, dtype=np.float32)

    for sb in range(GSB):
        B = int(B_sb[sb])
        w1 = _win_sched(B)
        base = blk_of_sb[sb] * 128
        firsts = [int(np.where((w1 == v) | (w1 == v - 1))[0][0]) for v in range(SBW)]
        for c in range(C):
            pos = 0
            for v in range(SBW):
                gidx = sb * SBW + v
                n = cnt_cw[c, gidx]
                pos = max(pos, firsts[v] * 128)
                if n:
                    e0 = flat_start[c * GW + gidx]
                    sl = slice(base + pos, base + pos + n)
                    idx_slot[c, sl] = srow_s[e0:e0 + n]
                    w_slot[c, sl] = w_s[e0:e0 + n]
                    win_slot[c, sl] = gidx
                    lane_slot[c, sl] = lane_s[e0:e0 + n]
                    pos += n

    prim_w1 = np.empty(NBLK, dtype=np.int64)
    for sb in range(GSB):
        B = int(B_sb[sb])
        w1 = _win_sched(B)
        prim_w1[blk_of_sb[sb]:blk_of_sb[sb] + B] = sb * SBW + w1

    sec_needed = np.zeros(NBLK, dtype=bool)
    blk_of_slot = np.arange(TOT) // 128
    for c in range(C):
        m = win_slot[c] >= 0
        sec = win_slot[c][m] != prim_w1[blk_of_slot[m]]
        np.logical_or.at(sec_needed, blk_of_slot[m][sec], True)

    mm_block, mm_win = [], []
    for blk in range(NBLK):
        mm_block.append(blk); mm_win.append(int(prim_w1[blk]))
        if sec_needed[blk]:
            mm_block.append(blk); mm_win.append(int(prim_w1[blk]) + 1)
    mm_block = np.array(mm_block); mm_win = np.array(mm_win)
    NMM = len(mm_block)

    first_mm, last_mm = {}, {}
    for m in range(NMM):
        wn = int(mm_win[m])
        if wn not in first_mm:
            first_mm[wn] = m
        last_mm[wn] = m
    assert len(first_mm) == GW

    ls = lane_slot.reshape(C, NBLK, 128)
    vs = win_slot.reshape(C, NBLK, 128)
    dstl_mm = np.full((C, 128, NMM), -1.0, dtype=np.float32)
    for m in range(NMM):
        blk, wn = int(mm_block[m]), int(mm_win[m])
        sel = vs[:, blk, :] == wn
        dstl_mm[:, :, m] = np.where(sel, ls[:, blk, :], -1.0)
    wsl_blk = w_slot.reshape(C, NBLK, 128).transpose(0, 2, 1).copy()

    n_ops = (TOT + OPN - 1) // OPN
    op_sizes = [min(OPN, TOT - i * OPN) for i in range(n_ops)]
    idx_wrap = np.zeros((C, 16, TOT // 16), dtype=np.int16)
    off = 0
    for s in op_sizes:
        seg = idx_slot[:, off:off + s].reshape(C, s // 16, 16)
        idx_wrap[:, :, off // 16:(off + s) // 16] = seg.transpose(0, 2, 1)
        off += s
    idx_full = np.tile(idx_wrap, (1, 8, 1))

    o = np.arange(NODES_C)
    u_of = o // 128
    p_of = o % 128
    dinv_lane = np.ones((C, 128, W), dtype=np.float32)
    bl_lane = np.full((C, 128, W), 63.0, dtype=np.float32)
    gmin = np.zeros(C, dtype=np.int64)
    xT = np.zeros((C, IN_DIM, PADN), dtype=np.float32)
    for c in range(C):
        n0 = c * NODES_C
        dinv_lane[c, p_of, u_of] = dinv_full[n0:n0 + NODES_C]
        bseg = batch[n0:n0 + NODES_C]
        gmin[c] = bseg[0]
        assert int(bseg[-1] - bseg[0]) <= 62
        bl_lane[c, p_of, u_of] = (bseg - gmin[c]).astype(np.float32)
        xT[c, :, :NODES_C] = x[n0:n0 + NODES_C].T

    return dict(
        B_sb=B_sb, blk_of_sb=blk_of_sb, NBLK=NBLK, TOT=TOT, NMM=NMM,
        mm_block=mm_block, mm_win=mm_win, first_mm=first_mm, last_mm=last_mm,
        dstl_mm=dstl_mm, wsl_blk=wsl_blk, idx_full=idx_full,
        dinv_lane=dinv_lane, bl_lane=bl_lane, gmin=gmin, xT=xT,
        op_sizes=op_sizes,
    )


# --------------------------------------------------------------------------
def _build_program(prep, has_bias=True):
    import concourse.bacc as bacc
    import concourse.mybir as mybir
    import concourse.tile as tile
    from contextlib import ExitStack

    f32 = mybir.dt.float32
    bf16 = mybir.dt.bfloat16
    i16 = mybir.dt.int16
    OP = mybir.AluOpType
    AF = mybir.ActivationFunctionType

    NBLK = prep["NBLK"]
    TOT = prep["TOT"]
    NMM = prep["NMM"]
    mm_block = prep["mm_block"]
    mm_win = prep["mm_win"]
    first_mm = prep["first_mm"]
    last_mm = prep["last_mm"]
    op_sizes = prep["op_sizes"]
    blk_of_sb = prep["blk_of_sb"]

    # sb index of each global window; last window of each sb
    sb_of_win = np.arange(GW) // SBW

    nc = bacc.Bacc("TRN2", target_bir_lowering=False, debug=False, num_devices=C)

    xT_in = nc.declare_dram_parameter("xT", [IN_DIM, PADN], bf16, isOutput=False)
    W1_in = nc.declare_dram_parameter("W1", [IN_DIM, F], bf16, isOutput=False)
    W2_in = nc.declare_dram_parameter("W2", [F, F], bf16, isOutput=False)
    W3_in = nc.declare_dram_parameter("W3", [F, F], bf16, isOutput=False)
    ball_in = nc.declare_dram_parameter("ballw", [128, 3 * F], bf16, isOutput=False)
    iota128_in = nc.declare_dram_parameter("iota128", [128, 128], bf16, isOutput=False)
    iota64_in = nc.declare_dram_parameter("iota64", [128, F], bf16, isOutput=False)
    ident_in = nc.declare_dram_parameter("ident", [128, 128], bf16, isOutput=False)
    ones_in = nc.declare_dram_parameter("onescol", [128, 1], bf16, isOutput=False)
    dstl_in = nc.declare_dram_parameter("dstl", [128, NMM], f32, isOutput=False)
    wsl_in = nc.declare_dram_parameter("wsl", [128, NBLK], f32, isOutput=False)
    idx_in = nc.declare_dram_parameter("idx16", [128, TOT // 16], i16, isOutput=False)
    dinv_in = nc.declare_dram_parameter("dinv", [128, W], f32, isOutput=False)
    dinvw_in = nc.declare_dram_parameter("dinvwide", [128, W * F], bf16, isOutput=False)
    bl_in = nc.declare_dram_parameter("batchloc", [128, W], f32, isOutput=False)
    pool_out = nc.declare_dram_parameter("pool_out", [F, F + 1], f32, isOutput=True)

    stk = ExitStack()
    tbl_sems = [stk.enter_context(nc.semaphore(f"tbl_{i}")) for i in range(3)]
    wr_sems = [stk.enter_context(nc.semaphore(f"wr_{i}")) for i in range(3)]
    cc_sems = [stk.enter_context(nc.semaphore(f"cc_{i}")) for i in range(3)]

    n_sb_dma = GSB  # staging DMAs per layer

    with tile.TileContext(nc, num_cores=C) as tc:
        tc.race_detector_enabled = False
        with (
            tc.tile_pool(name="persist", bufs=1) as pp,
            tc.tile_pool(name="idxp", bufs=3) as idxp,
            tc.tile_pool(name="msgp", bufs=3) as mp,
            tc.tile_pool(name="wstp", bufs=8) as wp,
            tc.tile_pool(name="stgp", bufs=3) as sgp,
            tc.tile_pool(name="epi", bufs=1) as ep,
            tc.tile_pool(name="ps", bufs=7, space="PSUM") as ps,
            tc.tile_pool(name="psPool", bufs=1, space="PSUM") as ps1,
            tc.tile_pool(name="dram", bufs=1, space="DRAM") as dr,
        ):
            def load(name, shape, dt, src):
                t = pp.tile(shape, dt, name=name)
                nc.sync.dma_start(out=t[:], in_=src[:])
                return t

            xT_a = load("xT_a", [IN_DIM, PADN], bf16, xT_in)
            w1 = load("w1", [IN_DIM, F], bf16, W1_in)
            w2 = load("w2", [F, F], bf16, W2_in)
            w3 = load("w3", [F, F], bf16, W3_in)
            ballw = load("ballw", [128, 3 * F], bf16, ball_in)
            iota128 = load("iota128", [128, 128], bf16, iota128_in)
            iota64 = load("iota64", [128, F], bf16, iota64_in)
            ident = load("ident", [128, 128], bf16, ident_in)
            onescol = load("onescol", [128, 1], bf16, ones_in)
            dstl = load("dstl", [128, NMM], f32, dstl_in)
            wsl = load("wsl", [128, NBLK], f32, wsl_in)
            dinv = load("dinv", [128, W], f32, dinv_in)
            dinvw = load("dinvwide", [128, W * F], bf16, dinvw_in)
            batchloc = load("batchloc", [128, W], f32, bl_in)

            ytb = pp.tile([128, W * 128], bf16, name="ytb")
            xTn = pp.tile([F, PADN], bf16, name="xTn")
            acc = pp.tile([128, W * F], bf16, name="acc")
            xpr = pp.tile([128, W * F], bf16, name="xpr")
            rs_sb = pp.tile([128, W * F], bf16, name="rs_sb")

            for L in range(3):
                wmat = (w1, w2, w3)[L]

                # ---------------- phase A: y~ = dinv * (x @ W), dup bf16 ----
                for u in range(W):
                    psum_y = ps.tile([128, F], f32, name="psum_y", tag="psum_y",
                                     bufs=3)
                    lhsT = (xT_a if L == 0 else xTn)[:, u * 128:(u + 1) * 128]
                    nc.tensor.matmul(psum_y[:], lhsT, wmat[:], start=True, stop=True)
                    nc.vector.tensor_scalar(
                        ytb[:, u * 128:u * 128 + F], psum_y[:],
                        dinv[:, u:u + 1], None, OP.mult)
                    nc.scalar.activation(
                        ytb[:, u * 128 + F:(u + 1) * 128], psum_y[:],
                        AF.Copy, scale=dinv[:, u:u + 1])

                # table write (rows r = p*W+u, 256B each, contiguous per p)
                table = dr.tile([PADN, 128], bf16, name=f"table_{L}")
                nc.sync.dma_start(
                    out=table[:], in_=ytb[:].rearrange("p (u e) -> (p u) e", e=128)
                ).then_inc(tbl_sems[L], 16)

                partial = dr.tile([C * PADN, F], bf16, name=f"partial_{L}")
                rsout = dr.tile([PADN, F], bf16, name=f"rsout_{L}")

                nc.gpsimd.wait_ge(tbl_sems[L], 16)

                # ---------------- gathers + block matmuls -------------------
                n_ops = len(op_sizes)
                mts = [None] * n_ops
                op_base_blk = [0] * n_ops
                off = 0
                for k, s in enumerate(op_sizes):
                    op_base_blk[k] = off // 128
                    off += s

                size_regs = {}
                for s in set(op_sizes):
                    size_regs[s] = nc.gpsimd.to_reg(s)

                def issue_gather(k):
                    s = op_sizes[k]
                    off16 = sum(op_sizes[:k]) // 16
                    it = idxp.tile([128, OPN // 16], i16, name="idxt", tag="idxt")
                    nc.sync.dma_start(out=it[:, :s // 16],
                                      in_=idx_in[:, off16:off16 + s // 16])
                    mt = mp.tile([128, OPN // 128, 128], bf16, name="msg", tag="msg")
                    nc.gpsimd.dma_gather(
                        out_ap=mt[:, :s // 128, :],
                        in_ap=table[:],
                        idxs_ap=it[:, :s // 16],
                        num_idxs=s,
                        num_idxs_reg=size_regs[s],
                        elem_size=128,
                    )
                    mts[k] = mt

                issue_gather(0)
                if n_ops > 1:
                    issue_gather(1)

                psums = {}
                stg = {}
                build_ctr = 0
                cur_op = 0
                for m in range(NMM):
                    blk = int(mm_block[m])
                    wn = int(mm_win[m])
                    # advance gather op when first matmul touches its blocks
                    while cur_op + 1 < n_ops and blk >= op_base_blk[cur_op + 1]:
                        cur_op += 1
                    for ahead in (1, 2):
                        if cur_op + ahead < n_ops and mts[cur_op + ahead] is None:
                            issue_gather(cur_op + ahead)
                    mt = mts[cur_op]
                    jloc = blk - op_base_blk[cur_op]

                    wst = wp.tile([128, 128], bf16, name="wst", tag="wst")
                    eng = nc.gpsimd if (build_ctr % 10) < int(POOL_BUILD_FRAC * 10) \
                        else nc.vector
                    eng.tensor_scalar(
                        wst[:], iota128[:], dstl[:, m:m + 1], wsl[:, blk:blk + 1],
                        OP.is_equal, OP.mult)
                    build_ctr += 1

                    if wn not in psums:
                        psums[wn] = ps.tile([128, F], f32, name="psum_w",
                                            tag="psum_w", bufs=3)
                    nc.tensor.matmul(
                        psums[wn][:], wst[:], mt[:, jloc, 0:F],
                        start=(m == first_mm[wn]), stop=(m == last_mm[wn]),
                        skip_group_check=True)

                    if m == last_mm[wn]:
                        sb = int(sb_of_win[wn])
                        v = wn % SBW
                        if sb not in stg:
                            stg[sb] = sgp.tile([128, SBW * F], bf16, name="stg",
                                               tag="stg")
                        nc.scalar.activation(
                            stg[sb][:, v * F:(v + 1) * F], psums.pop(wn)[:],
                            AF.Copy)
                        if v == SBW - 1:
                            kk = sb // NSB
                            u0 = (sb % NSB) * SBW
                            dst_ap = partial[:].rearrange(
                                "(k p u) f -> k p (u f)", k=C, p=128
                            )[kk][:, u0 * F:(u0 + SBW) * F]
                            nc.sync.dma_start(
                                out=dst_ap, in_=stg.pop(sb)[:]
                            ).then_inc(wr_sems[L], 16)

                # ---------------- ReduceScatter -----------------------------
                nc.gpsimd.wait_ge(wr_sems[L], 16 * n_sb_dma)
                nc.gpsimd.collective_compute(
                    "ReduceScatter",
                    OP.add,
                    replica_groups=[list(range(C))],
                    ins=[partial[:]],
                    outs=[rsout[:]],
                ).then_inc(cc_sems[L], 1)
                nc.gpsimd.wait_ge(cc_sems[L], 1)
                nc.gpsimd.dma_start(
                    out=rs_sb[:],
                    in_=rsout[:].rearrange("(p u) f -> p (u f)", p=128))

                # ---------------- wide epilogue -----------------------------
                ytb_self = ytb[:].rearrange("p (u e) -> p u e", e=128)[:, :, 0:F]
                nc.vector.tensor_tensor(xpr[:], rs_sb[:], ytb_self, OP.add)
                nc.vector.tensor_tensor(xpr[:], xpr[:], dinvw[:], OP.mult)
                if has_bias:
                    for u in range(W):
                        nc.vector.tensor_tensor(
                            xpr[:, u * F:(u + 1) * F], xpr[:, u * F:(u + 1) * F],
                            ballw[:, L * F:(L + 1) * F], OP.add)
                nc.vector.tensor_scalar(xpr[:], xpr[:], 0.0, None, OP.max)
                if L == 0:
                    nc.vector.tensor_copy(acc[:], xpr[:])
                else:
                    nc.vector.tensor_tensor(acc[:], acc[:], xpr[:], OP.add)

                if L < 2:
                    for u in range(W):
                        ptr = ps.tile([F, 128], bf16, name="ptr", tag="ptr",
                                      bufs=1)
                        nc.tensor.transpose(ptr[:], xpr[:, u * F:(u + 1) * F],
                                            ident[:])
                        eng = nc.gpsimd if u % 2 == 0 else nc.vector
                        eng.tensor_copy(xTn[:, u * 128:(u + 1) * 128], ptr[:])

            # ---------------- pooling -----------------------------------
            psum_pool = ps1.tile([F, F + 1], f32, name="psum_pool")
            psum_sums = psum_pool[:, 0:F]
            psum_cnt = psum_pool[:, F:F + 1]
            for u in range(W):
                sg = wp.tile([128, F], bf16, name="sg", tag="sg")
                nc.vector.tensor_scalar(
                    sg[:], iota64[:], batchloc[:, u:u + 1], None, OP.is_equal)
                nc.tensor.matmul(
                    psum_sums, sg[:], acc[:, u * F:(u + 1) * F],
                    start=(u == 0), stop=(u == W - 1), skip_group_check=True)
                nc.tensor.matmul(
                    psum_cnt, sg[:], onescol[:],
                    start=(u == 0), stop=(u == W - 1), skip_group_check=True)
            outt = ep.tile([F, F + 1], f32, name="outt")
            nc.vector.tensor_copy(outt[:, :F], psum_sums)
            nc.vector.tensor_copy(outt[:, F:F + 1], psum_cnt)
            nc.sync.dma_start(out=pool_out[:], in_=outt[:])

    stk.close()
    nc.compile()
    _split_waits(nc)
    return nc


# --------------------------------------------------------------------------
def kernel(x, edge_weight, W1, b1, W2, b2, W3, b3, Wl, bl, edge_index, batch):
    from concourse.bass_utils import run_bass_kernel_spmd
    import jax.numpy as jnp

    prep = _host_prep(x, edge_weight, edge_index, batch)

    has_bias = any(np.any(np.asarray(b) != 0) for b in (b1, b2, b3))
    cache_key = (prep["NBLK"], prep["NMM"], tuple(prep["op_sizes"][:3]), has_bias)
    if cache_key not in _prog_cache:
        _prog_cache[cache_key] = _build_program(prep, has_bias=has_bias)
    nc = _prog_cache[cache_key]

    bf = lambda a: np.asarray(jnp.asarray(np.asarray(a, np.float32), jnp.bfloat16))
    W1b, W2b, W3b = bf(W1), bf(W2), bf(W3)
    ballw = np.zeros((128, 3 * F), dtype=np.float32)
    ballw[:, 0:F] = np.asarray(b1, np.float32)[None, :]
    ballw[:, F:2 * F] = np.asarray(b2, np.float32)[None, :]
    ballw[:, 2 * F:3 * F] = np.asarray(b3, np.float32)[None, :]
    ballw = bf(ballw)
    iota128 = bf(np.tile(np.arange(128, dtype=np.float32)[None, :], (128, 1)))
    iota64 = bf(np.tile(np.arange(F, dtype=np.float32)[None, :], (128, 1)))
    ident = bf(np.eye(128, dtype=np.float32))
    onescol = bf(np.ones((128, 1), dtype=np.float32))
    xT_bf = bf(prep["xT"])
    dinvw = bf(np.repeat(prep["dinv_lane"], F, axis=2))  # [C,128,W*F]

    in_maps = []
    for c in range(C):
        in_maps.append({
            "xT": xT_bf[c],
            "W1": W1b, "W2": W2b, "W3": W3b, "ballw": ballw,
            "iota128": iota128, "iota64": iota64, "ident": ident,
            "onescol": onescol,
            "dstl": prep["dstl_mm"][c], "wsl": prep["wsl_blk"][c],
            "idx16": prep["idx_full"][c],
            "dinv": prep["dinv_lane"][c], "dinvwide": dinvw[c],
            "batchloc": prep["bl_lane"][c],
        })

    res = run_bass_kernel_spmd(nc, in_maps, core_ids=list(range(C)))

    sums = np.zeros((N_GRAPHS, F), dtype=np.float64)
    cnts = np.zeros(N_GRAPHS, dtype=np.float64)
    for c in range(C):
        out = res.results[c]["pool_out"]
        g0 = int(prep["gmin"][c])
        for r in range(63):
            g = g0 + r
            if g < N_GRAPHS:
                sums[g] += out[r, :F]
                cnts[g] += out[r, F]
    pooled = (sums / 3.0) / np.maximum(cnts, 1.0)[:, None]
    logits = pooled @ np.asarray(Wl, np.float64) + np.asarray(bl, np.float64)
    z = logits - logits.max(axis=1, keepdims=True)
    ez = np.exp(z)
    return (ez / ez.sum(axis=1, keepdims=True)).astype(np.float32)


# revision 31
# speedup vs baseline: 1.2433x; 1.2433x over previous
"""GCN (3-layer + mean-pool + linear + softmax) on 8 Trainium2 NeuronCores.

Push-mode graph parallelism: each core owns a contiguous 12,500-node range
(padded 12544 = 128 lanes x 98 windows). Per layer, each core:
  phase A:  y~ = dinv * (x @ W) on PE, written as a bf16 table with duplicated
            rows [y~|y~] (256B rows) to DRAM,
  gathers   its OWN out-edge sources from the local table (int16 row ids,
            single chunk, large batched dma_gather ops),
  scatters  messages into per-global-window PSUM accumulators via fused
            one-hot matmuls: lhsT = (iota==dstlane)*|w| built in one
            tensor_scalar(is_equal,mult) op (DVE 4x mode / gpsimd),
  writes    bf16 partial sums [100352, 64] (partition-major rows -> large
            contiguous DMA descriptors),
  ReduceScatter (out 12544x64 bf16 ~= 55us) delivers summed aggregates for its
            own nodes; wide fused epilogue: x' = relu(dinv*(rs + y~self) + b).
Static SPMD schedule: superblocks of 7 windows with ~15 blocks each; blocks
serve a primary window and optionally the next (straddle), absorbing per-core
count variation with <=7% slot padding. Pooling via one-hot(graph) matmuls;
host applies the final 64x10 linear + softmax.
"""
import os
import sys
import numpy as np

sys.path.insert(0, os.path.dirname(os.path.abspath(__file__)))

N_NODES = 100000
N_GRAPHS = 256
IN_DIM = 128
F = 64
C = 8
NODES_C = 12500
PADN = 12544
W = 98
SBW = 7
NSB = W // SBW
GW = C * W
GSB = C * NSB
OPN = 8192
POOL_BUILD_FRAC = 0.0    # fraction of one-hot builds on gpsimd (Pool)

_prog_cache = {}


def _win_sched(B):
    return np.minimum((np.arange(B) * SBW) // B, SBW - 1)


# --------------------------------------------------------------------------
def _split_waits(nc, cap=1):
    """Walrus rejects >1 sem wait per instruction; hoist extras onto injected
    same-engine InstEventSemaphore waits."""
    import concourse.mybir as mybir
    uid = [0]
    n_fixed = 0
    for fn in nc.m.functions:
        for bb in fn.blocks:
            insts = bb.instructions
            new_list = []
            for inst in insts:
                si = inst.sync_info
                waits = list(si.on_wait) if si and si.on_wait else []
                if len(waits) > cap:
                    extra, keep = waits[:-cap], waits[-cap:]
                    for wv in extra:
                        uid[0] += 1
                        nop = mybir.InstEventSemaphore(name=f"waitfix_{uid[0]}")
                        nop.engine = inst.engine
                        nop.sync_info = mybir.SyncInfo(on_wait=[wv], on_update=[])
                        new_list.append(nop)
                    si.on_wait = keep
                    n_fixed += 1
                new_list.append(inst)
            if len(new_list) != len(insts):
                try:
                    bb.instructions = new_list
                except Exception:
                    insts.clear()
                    insts.extend(new_list)
    return n_fixed


# --------------------------------------------------------------------------
def _host_prep(x, edge_weight, edge_index, batch):
    src = np.asarray(edge_index[0], dtype=np.int64)
    dst = np.asarray(edge_index[1], dtype=np.int64)
    w_abs = np.abs(np.asarray(edge_weight, dtype=np.float32))
    batch = np.asarray(batch, dtype=np.int64)
    x = np.asarray(x, dtype=np.float32)

    deg = np.bincount(dst, weights=w_abs.astype(np.float64), minlength=N_NODES) + 1.0
    dinv_full = (1.0 / np.sqrt(deg)).astype(np.float32)

    ks = src // NODES_C
    so = src - ks * NODES_C
    srow = ((so % 128) * W + so // 128).astype(np.int16)
    kd = dst // NODES_C
    do = dst - kd * NODES_C
    lane = (do % 128).astype(np.float32)
    g = kd * W + do // 128

    gsb = g // SBW
    key = (ks * GSB + gsb) * SBW + (g % SBW)
    order = np.argsort(key, kind="stable")
    ks_s, g_s = ks[order], g[order]
    srow_s, lane_s, w_s = srow[order], lane[order], w_abs[order]

    cnt_cw = np.zeros((C, GW), dtype=np.int64)
    np.add.at(cnt_cw, (ks_s, g_s), 1)
    cnt_csb = cnt_cw.reshape(C, GSB, SBW).sum(axis=2)

    B_sb = np.maximum(np.ceil(cnt_csb.max(axis=0) / 128).astype(np.int64), SBW)

    def feasible(sb, B):
        w1 = _win_sched(B)
        firsts = [np.where((w1 == v) | (w1 == v - 1))[0][0] for v in range(SBW)]
        lasts = [np.where(w1 == v)[0][-1] for v in range(SBW)]
        for c in range(C):
            pos = 0
            for v in range(SBW):
                pos = max(pos, int(firsts[v]) * 128)
                pos += cnt_cw[c, sb * SBW + v]
                if pos > (int(lasts[v]) + 1) * 128:
                    return False
        return True

    for sb in range(GSB):
        while not feasible(sb, int(B_sb[sb])):
            B_sb[sb] += 1

    blk_of_sb = np.concatenate([[0], np.cumsum(B_sb)])
    NBLK = int(blk_of_sb[-1])
    TOT = NBLK * 128

    flat_cnt = np.zeros(C * GW, dtype=np.int64)
    np.add.at(flat_cnt, ks_s * GW + g_s, 1)
    flat_start = np.concatenate([[0], np.cumsum(flat_cnt)])

    idx_slot = np.zeros((C, TOT), dtype=np.int16)
    w_slot = np.zeros((C, TOT), dtype=np.float32)
    win_slot = np.full((C, TOT), -1, dtype=np.int64)
    lane_slot = np.full((C, TOT), -1.0, dtype=np.float32)

    for sb in range(GSB):
        B = int(B_sb[sb])
        w1 = _win_sched(B)
        base = blk_of_sb[sb] * 128
        firsts = [int(np.where((w1 == v) | (w1 == v - 1))[0][0]) for v in range(SBW)]
        for c in range(C):
            pos = 0
            for v in range(SBW):
                gidx = sb * SBW + v
                n = cnt_cw[c, gidx]
                pos = max(pos, firsts[v] * 128)
                if n:
                    e0 = flat_start[c * GW + gidx]
                    sl = slice(base + pos, base + pos + n)
                    idx_slot[c, sl] = srow_s[e0:e0 + n]
                    w_slot[c, sl] = w_s[e0:e0 + n]
                    win_slot[c, sl] = gidx
                    lane_slot[c, sl] = lane_s[e0:e0 + n]
                    pos += n

    prim_w1 = np.empty(NBLK, dtype=np.int64)
    for sb in range(GSB):
        B = int(B_sb[sb])
        w1 = _win_sched(B)
        prim_w1[blk_of_sb[sb]:blk_of_sb[sb] + B] = sb * SBW + w1

    sec_needed = np.zeros(NBLK, dtype=bool)
    blk_of_slot = np.arange(TOT) // 128
    for c in range(C):
        m = win_slot[c] >= 0
        sec = win_slot[c][m] != prim_w1[blk_of_slot[m]]
        np.logical_or.at(sec_needed, blk_of_slot[m][sec], True)

    mm_block, mm_win = [], []
    for blk in range(NBLK):
        mm_block.append(blk); mm_win.append(int(prim_w1[blk]))
        if sec_needed[blk]:
            mm_block.append(blk); mm_win.append(int(prim_w1[blk]) + 1)
    mm_block = np.array(mm_block); mm_win = np.array(mm_win)
    NMM = len(mm_block)

    first_mm, last_mm = {}, {}
    for m in range(NMM):
        wn = int(mm_win[m])
        if wn not in first_mm:
            first_mm[wn] = m
        last_mm[wn] = m
    assert len(first_mm) == GW

    ls = lane_slot.reshape(C, NBLK, 128)
    vs = win_slot.reshape(C, NBLK, 128)
    dstl_mm = np.full((C, 128, NMM), -1.0, dtype=np.float32)
    for m in range(NMM):
        blk, wn = int(mm_block[m]), int(mm_win[m])
        sel = vs[:, blk, :] == wn
        dstl_mm[:, :, m] = np.where(sel, ls[:, blk, :], -1.0)
    wsl_blk = w_slot.reshape(C, NBLK, 128).transpose(0, 2, 1).copy()

    n_ops = (TOT + OPN - 1) // OPN
    op_sizes = [min(OPN, TOT - i * OPN) for i in range(n_ops)]
    idx_wrap = np.zeros((C, 16, TOT // 16), dtype=np.int16)
    off = 0
    for s in op_sizes:
        seg = idx_slot[:, off:off + s].reshape(C, s // 16, 16)
        idx_wrap[:, :, off // 16:(off + s) // 16] = seg.transpose(0, 2, 1)
        off += s
    idx_full = np.tile(idx_wrap, (1, 8, 1))

    o = np.arange(NODES_C)
    u_of = o // 128
    p_of = o % 128
    dinv_lane = np.ones((C, 128, W), dtype=np.float32)
    bl_lane = np.full((C, 128, W), 63.0, dtype=np.float32)
    gmin = np.zeros(C, dtype=np.int64)
    xT = np.zeros((C, IN_DIM, PADN), dtype=np.float32)
    for c in range(C):
        n0 = c * NODES_C
        dinv_lane[c, p_of, u_of] = dinv_full[n0:n0 + NODES_C]
        bseg = batch[n0:n0 + NODES_C]
        gmin[c] = bseg[0]
        assert int(bseg[-1] - bseg[0]) <= 62
        bl_lane[c, p_of, u_of] = (bseg - gmin[c]).astype(np.float32)
        xT[c, :, :NODES_C] = x[n0:n0 + NODES_C].T

    return dict(
        B_sb=B_sb, blk_of_sb=blk_of_sb, NBLK=NBLK, TOT=TOT, NMM=NMM,
        mm_block=mm_block, mm_win=mm_win, first_mm=first_mm, last_mm=last_mm,
        dstl_mm=dstl_mm, wsl_blk=wsl_blk, idx_full=idx_full,
        dinv_lane=dinv_lane, bl_lane=bl_lane, gmin=gmin, xT=xT,
        op_sizes=op_sizes,
    )


# --------------------------------------------------------------------------
def _build_program(prep, has_bias=True):
    import concourse.bacc as bacc
    import concourse.mybir as mybir
    import concourse.tile as tile
    from contextlib import ExitStack

    f32 = mybir.dt.float32
    bf16 = mybir.dt.bfloat16
    i16 = mybir.dt.int16
    OP = mybir.AluOpType
    AF = mybir.ActivationFunctionType

    NBLK = prep["NBLK"]
    TOT = prep["TOT"]
    NMM = prep["NMM"]
    mm_block = prep["mm_block"]
    mm_win = prep["mm_win"]
    first_mm = prep["first_mm"]
    last_mm = prep["last_mm"]
    op_sizes = prep["op_sizes"]
    blk_of_sb = prep["blk_of_sb"]

    # sb index of each global window; last window of each sb
    sb_of_win = np.arange(GW) // SBW

    nc = bacc.Bacc("TRN2", target_bir_lowering=False, debug=False, num_devices=C)

    xT_in = nc.declare_dram_parameter("xT", [IN_DIM, PADN], bf16, isOutput=False)
    W1_in = nc.declare_dram_parameter("W1", [IN_DIM, F], bf16, isOutput=False)
    W2_in = nc.declare_dram_parameter("W2", [F, F], bf16, isOutput=False)
    W3_in = nc.declare_dram_parameter("W3", [F, F], bf16, isOutput=False)
    ball_in = nc.declare_dram_parameter("ballw", [128, 3 * F], bf16, isOutput=False)
    iota128_in = nc.declare_dram_parameter("iota128", [128, 128], bf16, isOutput=False)
    iota64_in = nc.declare_dram_parameter("iota64", [128, F], bf16, isOutput=False)
    ident_in = nc.declare_dram_parameter("ident", [128, 128], bf16, isOutput=False)
    ones_in = nc.declare_dram_parameter("onescol", [128, 1], bf16, isOutput=False)
    dstl_in = nc.declare_dram_parameter("dstl", [128, NMM], f32, isOutput=False)
    wsl_in = nc.declare_dram_parameter("wsl", [128, NBLK], f32, isOutput=False)
    idx_in = nc.declare_dram_parameter("idx16", [128, TOT // 16], i16, isOutput=False)
    dinv_in = nc.declare_dram_parameter("dinv", [128, W], f32, isOutput=False)
    dinvw_in = nc.declare_dram_parameter("dinvwide", [128, W * F], bf16, isOutput=False)
    bl_in = nc.declare_dram_parameter("batchloc", [128, W], f32, isOutput=False)
    pool_out = nc.declare_dram_parameter("pool_out", [F, F + 1], f32, isOutput=True)

    stk = ExitStack()
    tbl_sems = [stk.enter_context(nc.semaphore(f"tbl_{i}")) for i in range(3)]
    wr_sems = [stk.enter_context(nc.semaphore(f"wr_{i}")) for i in range(3)]
    cc_sems = [stk.enter_context(nc.semaphore(f"cc_{i}")) for i in range(3)]

    n_sb_dma = GSB // 2  # staging DMAs per layer (2 superblocks each)

    with tile.TileContext(nc, num_cores=C) as tc:
        tc.race_detector_enabled = False
        with (
            tc.tile_pool(name="persist", bufs=1) as pp,
            tc.tile_pool(name="idxp", bufs=3) as idxp,
            tc.tile_pool(name="msgp", bufs=3) as mp,
            tc.tile_pool(name="wstp", bufs=16) as wp,
            tc.tile_pool(name="stgp", bufs=3) as sgp,
            tc.tile_pool(name="epi", bufs=1) as ep,
            tc.tile_pool(name="ps", bufs=7, space="PSUM") as ps,
            tc.tile_pool(name="psPool", bufs=1, space="PSUM") as ps1,
            tc.tile_pool(name="dram", bufs=1, space="DRAM") as dr,
        ):
            def load(name, shape, dt, src):
                t = pp.tile(shape, dt, name=name)
                nc.sync.dma_start(out=t[:], in_=src[:])
                return t

            xT_a = load("xT_a", [IN_DIM, PADN], bf16, xT_in)
            w1 = load("w1", [IN_DIM, F], bf16, W1_in)
            w2 = load("w2", [F, F], bf16, W2_in)
            w3 = load("w3", [F, F], bf16, W3_in)
            ballw = load("ballw", [128, 3 * F], bf16, ball_in)
            iota128 = load("iota128", [128, 128], bf16, iota128_in)
            iota64 = load("iota64", [128, F], bf16, iota64_in)
            ident = load("ident", [128, 128], bf16, ident_in)
            onescol = load("onescol", [128, 1], bf16, ones_in)
            dstl = load("dstl", [128, NMM], f32, dstl_in)
            wsl = load("wsl", [128, NBLK], f32, wsl_in)
            dinv = load("dinv", [128, W], f32, dinv_in)
            dinvw = load("dinvwide", [128, W * F], bf16, dinvw_in)
            batchloc = load("batchloc", [128, W], f32, bl_in)

            ytb = pp.tile([128, W * 128], bf16, name="ytb")
            xTn = pp.tile([F, PADN], bf16, name="xTn")
            acc = pp.tile([128, W * F], bf16, name="acc")
            xpr = pp.tile([128, W * F], bf16, name="xpr")
            rs_sb = pp.tile([128, W * F], bf16, name="rs_sb")

            for L in range(3):
                wmat = (w1, w2, w3)[L]

                # ---------------- phase A: y~ = dinv * (x @ W), dup bf16 ----
                for u in range(W):
                    psum_y = ps.tile([128, F], f32, name="psum_y", tag="psum_y",
                                     bufs=3)
                    lhsT = (xT_a if L == 0 else xTn)[:, u * 128:(u + 1) * 128]
                    nc.tensor.matmul(psum_y[:], lhsT, wmat[:], start=True, stop=True)
                    nc.vector.tensor_scalar(
                        ytb[:, u * 128:u * 128 + F], psum_y[:],
                        dinv[:, u:u + 1], None, OP.mult)
                    nc.scalar.activation(
                        ytb[:, u * 128 + F:(u + 1) * 128], psum_y[:],
                        AF.Copy, scale=dinv[:, u:u + 1])

                # table write (rows r = p*W+u, 256B each, contiguous per p)
                table = dr.tile([PADN, 128], bf16, name=f"table_{L}")
                nc.sync.dma_start(
                    out=table[:], in_=ytb[:].rearrange("p (u e) -> (p u) e", e=128)
                ).then_inc(tbl_sems[L], 16)

                partial = dr.tile([C * PADN, F], bf16, name=f"partial_{L}")
                rsout = dr.tile([PADN, F], bf16, name=f"rsout_{L}")

                nc.gpsimd.wait_ge(tbl_sems[L], 16)

                # ---------------- gathers + block matmuls -------------------
                n_ops = len(op_sizes)
                mts = [None] * n_ops
                op_base_blk = [0] * n_ops
                off = 0
                for k, s in enumerate(op_sizes):
                    op_base_blk[k] = off // 128
                    off += s

                size_regs = {}
                for s in set(op_sizes):
                    size_regs[s] = nc.gpsimd.to_reg(s)

                def issue_gather(k):
                    s = op_sizes[k]
                    off16 = sum(op_sizes[:k]) // 16
                    it = idxp.tile([128, OPN // 16], i16, name="idxt", tag="idxt")
                    nc.gpsimd.dma_start(out=it[:, :s // 16],
                                        in_=idx_in[:, off16:off16 + s // 16])
                    mt = mp.tile([128, OPN // 128, 128], bf16, name="msg", tag="msg")
                    nc.gpsimd.dma_gather(
                        out_ap=mt[:, :s // 128, :],
                        in_ap=table[:],
                        idxs_ap=it[:, :s // 16],
                        num_idxs=s,
                        num_idxs_reg=size_regs[s],
                        elem_size=128,
                    )
                    mts[k] = mt

                issue_gather(0)
                if n_ops > 1:
                    issue_gather(1)

                psums = {}
                stg = {}
                build_ctr = 0
                cur_op = 0
                for m in range(NMM):
                    blk = int(mm_block[m])
                    wn = int(mm_win[m])
                    # advance gather op when first matmul touches its blocks
                    while cur_op + 1 < n_ops and blk >= op_base_blk[cur_op + 1]:
                        cur_op += 1
                    for ahead in (1, 2):
                        if cur_op + ahead < n_ops and mts[cur_op + ahead] is None:
                            issue_gather(cur_op + ahead)
                    mt = mts[cur_op]
                    jloc = blk - op_base_blk[cur_op]

                    wst = wp.tile([128, 128], bf16, name="wst", tag="wst")
                    eng = nc.gpsimd if (build_ctr % 10) < int(POOL_BUILD_FRAC * 10) \
                        else nc.vector
                    eng.tensor_scalar(
                        wst[:], iota128[:], dstl[:, m:m + 1], wsl[:, blk:blk + 1],
                        OP.is_equal, OP.mult)
                    build_ctr += 1

                    if wn not in psums:
                        psums[wn] = ps.tile([128, F], f32, name="psum_w",
                                            tag="psum_w", bufs=3)
                    nc.tensor.matmul(
                        psums[wn][:], wst[:], mt[:, jloc, 0:F],
                        start=(m == first_mm[wn]), stop=(m == last_mm[wn]),
                        skip_group_check=True)

                    if m == last_mm[wn]:
                        sb2 = int(sb_of_win[wn]) // 2
                        v2 = wn % (2 * SBW)
                        if sb2 not in stg:
                            stg[sb2] = sgp.tile([128, 2 * SBW * F], bf16,
                                                name="stg", tag="stg")
                        nc.scalar.activation(
                            stg[sb2][:, v2 * F:(v2 + 1) * F], psums.pop(wn)[:],
                            AF.Copy)
                        if v2 == 2 * SBW - 1:
                            kk = wn // W
                            u0 = (wn % W) - (2 * SBW - 1)
                            dst_ap = partial[:].rearrange(
                                "(k p u) f -> k p (u f)", k=C, p=128
                            )[kk][:, u0 * F:(u0 + 2 * SBW) * F]
                            nc.sync.dma_start(
                                out=dst_ap, in_=stg.pop(sb2)[:]
                            ).then_inc(wr_sems[L], 16)

                # ---------------- ReduceScatter -----------------------------
                nc.gpsimd.wait_ge(wr_sems[L], 16 * n_sb_dma)
                nc.gpsimd.collective_compute(
                    "ReduceScatter",
                    OP.add,
                    replica_groups=[list(range(C))],
                    ins=[partial[:]],
                    outs=[rsout[:]],
                ).then_inc(cc_sems[L], 1)
                nc.gpsimd.wait_ge(cc_sems[L], 1)
                nc.gpsimd.dma_start(
                    out=rs_sb[:],
                    in_=rsout[:].rearrange("(p u) f -> p (u f)", p=128))

                # ---------------- wide epilogue -----------------------------
                ytb_self = ytb[:].rearrange("p (u e) -> p u e", e=128)[:, :, 0:F]
                nc.vector.tensor_tensor(xpr[:], rs_sb[:], ytb_self, OP.add)
                nc.vector.tensor_tensor(xpr[:], xpr[:], dinvw[:], OP.mult)
                if has_bias:
                    for u in range(W):
                        nc.vector.tensor_tensor(
                            xpr[:, u * F:(u + 1) * F], xpr[:, u * F:(u + 1) * F],
                            ballw[:, L * F:(L + 1) * F], OP.add)
                nc.vector.tensor_scalar(xpr[:], xpr[:], 0.0, None, OP.max)
                if L == 0:
                    nc.vector.tensor_copy(acc[:], xpr[:])
                else:
                    nc.vector.tensor_tensor(acc[:], acc[:], xpr[:], OP.add)

                if L < 2:
                    for u in range(W):
                        ptr = ps.tile([F, 128], bf16, name="ptr", tag="ptr",
                                      bufs=1)
                        nc.tensor.transpose(ptr[:], xpr[:, u * F:(u + 1) * F],
                                            ident[:])
                        eng = nc.gpsimd if u % 2 == 0 else nc.vector
                        eng.tensor_copy(xTn[:, u * 128:(u + 1) * 128], ptr[:])

            # ---------------- pooling -----------------------------------
            psum_pool = ps1.tile([F, F + 1], f32, name="psum_pool")
            psum_sums = psum_pool[:, 0:F]
            psum_cnt = psum_pool[:, F:F + 1]
            for u in range(W):
                sg = wp.tile([128, F], bf16, name="sg", tag="sg")
                nc.vector.tensor_scalar(
                    sg[:], iota64[:], batchloc[:, u:u + 1], None, OP.is_equal)
                nc.tensor.matmul(
                    psum_sums, sg[:], acc[:, u * F:(u + 1) * F],
                    start=(u == 0), stop=(u == W - 1), skip_group_check=True)
                nc.tensor.matmul(
                    psum_cnt, sg[:], onescol[:],
                    start=(u == 0), stop=(u == W - 1), skip_group_check=True)
            outt = ep.tile([F, F + 1], f32, name="outt")
            nc.vector.tensor_copy(outt[:, :F], psum_sums)
            nc.vector.tensor_copy(outt[:, F:F + 1], psum_cnt)
            nc.sync.dma_start(out=pool_out[:], in_=outt[:])

    stk.close()
    nc.compile()
    _split_waits(nc)
    return nc


# --------------------------------------------------------------------------
def kernel(x, edge_weight, W1, b1, W2, b2, W3, b3, Wl, bl, edge_index, batch):
    from concourse.bass_utils import run_bass_kernel_spmd
    import jax.numpy as jnp

    prep = _host_prep(x, edge_weight, edge_index, batch)

    has_bias = any(np.any(np.asarray(b) != 0) for b in (b1, b2, b3))
    cache_key = (prep["NBLK"], prep["NMM"], tuple(prep["op_sizes"][:3]), has_bias)
    if cache_key not in _prog_cache:
        _prog_cache[cache_key] = _build_program(prep, has_bias=has_bias)
    nc = _prog_cache[cache_key]

    bf = lambda a: np.asarray(jnp.asarray(np.asarray(a, np.float32), jnp.bfloat16))
    W1b, W2b, W3b = bf(W1), bf(W2), bf(W3)
    ballw = np.zeros((128, 3 * F), dtype=np.float32)
    ballw[:, 0:F] = np.asarray(b1, np.float32)[None, :]
    ballw[:, F:2 * F] = np.asarray(b2, np.float32)[None, :]
    ballw[:, 2 * F:3 * F] = np.asarray(b3, np.float32)[None, :]
    ballw = bf(ballw)
    iota128 = bf(np.tile(np.arange(128, dtype=np.float32)[None, :], (128, 1)))
    iota64 = bf(np.tile(np.arange(F, dtype=np.float32)[None, :], (128, 1)))
    ident = bf(np.eye(128, dtype=np.float32))
    onescol = bf(np.ones((128, 1), dtype=np.float32))
    xT_bf = bf(prep["xT"])
    dinvw = bf(np.repeat(prep["dinv_lane"], F, axis=2))  # [C,128,W*F]

    in_maps = []
    for c in range(C):
        in_maps.append({
            "xT": xT_bf[c],
            "W1": W1b, "W2": W2b, "W3": W3b, "ballw": ballw,
            "iota128": iota128, "iota64": iota64, "ident": ident,
            "onescol": onescol,
            "dstl": prep["dstl_mm"][c], "wsl": prep["wsl_blk"][c],
            "idx16": prep["idx_full"][c],
            "dinv": prep["dinv_lane"][c], "dinvwide": dinvw[c],
            "batchloc": prep["bl_lane"][c],
        })

    res = run_bass_kernel_spmd(nc, in_maps, core_ids=list(range(C)))

    sums = np.zeros((N_GRAPHS, F), dtype=np.float64)
    cnts = np.zeros(N_GRAPHS, dtype=np.float64)
    for c in range(C):
        out = res.results[c]["pool_out"]
        g0 = int(prep["gmin"][c])
        for r in range(63):
            g = g0 + r
            if g < N_GRAPHS:
                sums[g] += out[r, :F]
                cnts[g] += out[r, F]
    pooled = (sums / 3.0) / np.maximum(cnts, 1.0)[:, None]
    logits = pooled @ np.asarray(Wl, np.float64) + np.asarray(bl, np.float64)
    z = logits - logits.max(axis=1, keepdims=True)
    ez = np.exp(z)
    return (ez / ez.sum(axis=1, keepdims=True)).astype(np.float32)


# revision 37
# speedup vs baseline: 1.4047x; 1.1299x over previous
"""GCN (3-layer + mean-pool + linear + softmax) on 8 Trainium2 NeuronCores.

Push-mode graph parallelism: each core owns a contiguous 12,500-node range
(padded 12544 = 128 lanes x 98 windows). Per layer, each core:
  phase A:  y~ = dinv * (x @ W) on PE, written as a bf16 table with duplicated
            rows [y~|y~] (256B rows) to DRAM,
  gathers   its OWN out-edge sources from the local table (int16 row ids,
            single chunk, large batched dma_gather ops),
  scatters  messages into per-global-window PSUM accumulators via fused
            one-hot matmuls: lhsT = (iota==dstlane)*|w| built in one
            tensor_scalar(is_equal,mult) op (DVE 4x mode / gpsimd),
  writes    bf16 partial sums [100352, 64] (partition-major rows -> large
            contiguous DMA descriptors),
  ReduceScatter (out 12544x64 bf16 ~= 55us) delivers summed aggregates for its
            own nodes; wide fused epilogue: x' = relu(dinv*(rs + y~self) + b).
Static SPMD schedule: superblocks of 7 windows with ~15 blocks each; blocks
serve a primary window and optionally the next (straddle), absorbing per-core
count variation with <=7% slot padding. Pooling via one-hot(graph) matmuls;
host applies the final 64x10 linear + softmax.
"""
import os
import sys
import numpy as np

sys.path.insert(0, os.path.dirname(os.path.abspath(__file__)))

N_NODES = 100000
N_GRAPHS = 256
IN_DIM = 128
F = 64
C = 8
NODES_C = 12500
PADN = 12544
W = 98
SBW = 7
NSB = W // SBW
GW = C * W
GSB = C * NSB
OPN = 8192
POOL_BUILD_FRAC = 0.0    # fraction of one-hot builds on gpsimd (Pool)

_prog_cache = {}


def _win_sched(B):
    return np.minimum((np.arange(B) * SBW) // B, SBW - 1)


# --------------------------------------------------------------------------
def _split_waits(nc, cap=1):
    """Walrus rejects >1 sem wait per instruction; hoist extras onto injected
    same-engine InstEventSemaphore waits."""
    import concourse.mybir as mybir
    uid = [0]
    n_fixed = 0
    for fn in nc.m.functions:
        for bb in fn.blocks:
            insts = bb.instructions
            new_list = []
            for inst in insts:
                si = inst.sync_info
                waits = list(si.on_wait) if si and si.on_wait else []
                if len(waits) > cap:
                    extra, keep = waits[:-cap], waits[-cap:]
                    for wv in extra:
                        uid[0] += 1
                        nop = mybir.InstEventSemaphore(name=f"waitfix_{uid[0]}")
                        nop.engine = inst.engine
                        nop.sync_info = mybir.SyncInfo(on_wait=[wv], on_update=[])
                        new_list.append(nop)
                    si.on_wait = keep
                    n_fixed += 1
                new_list.append(inst)
            if len(new_list) != len(insts):
                try:
                    bb.instructions = new_list
                except Exception:
                    insts.clear()
                    insts.extend(new_list)
    return n_fixed


# --------------------------------------------------------------------------
def _host_prep(x, edge_weight, edge_index, batch):
    src = np.asarray(edge_index[0], dtype=np.int64)
    dst = np.asarray(edge_index[1], dtype=np.int64)
    w_abs = np.abs(np.asarray(edge_weight, dtype=np.float32))
    batch = np.asarray(batch, dtype=np.int64)
    x = np.asarray(x, dtype=np.float32)

    deg = np.bincount(dst, weights=w_abs.astype(np.float64), minlength=N_NODES) + 1.0
    dinv_full = (1.0 / np.sqrt(deg)).astype(np.float32)

    ks = src // NODES_C
    so = src - ks * NODES_C
    srow = ((so % 128) * W + so // 128).astype(np.int16)
    kd = dst // NODES_C
    do = dst - kd * NODES_C
    lane = (do % 128).astype(np.float32)
    g = kd * W + do // 128

    gsb = g // SBW
    key = (ks * GSB + gsb) * SBW + (g % SBW)
    order = np.argsort(key, kind="stable")
    ks_s, g_s = ks[order], g[order]
    srow_s, lane_s, w_s = srow[order], lane[order], w_abs[order]

    cnt_cw = np.zeros((C, GW), dtype=np.int64)
    np.add.at(cnt_cw, (ks_s, g_s), 1)
    cnt_csb = cnt_cw.reshape(C, GSB, SBW).sum(axis=2)

    B_sb = np.maximum(np.ceil(cnt_csb.max(axis=0) / 128).astype(np.int64), SBW)

    def feasible(sb, B):
        w1 = _win_sched(B)
        firsts = [np.where((w1 == v) | (w1 == v - 1))[0][0] for v in range(SBW)]
        lasts = [np.where(w1 == v)[0][-1] for v in range(SBW)]
        for c in range(C):
            pos = 0
            for v in range(SBW):
                pos = max(pos, int(firsts[v]) * 128)
                pos += cnt_cw[c, sb * SBW + v]
                if pos > (int(lasts[v]) + 1) * 128:
                    return False
        return True

    for sb in range(GSB):
        while not feasible(sb, int(B_sb[sb])):
            B_sb[sb] += 1

    blk_of_sb = np.concatenate([[0], np.cumsum(B_sb)])
    NBLK = int(blk_of_sb[-1])
    TOT = NBLK * 128

    flat_cnt = np.zeros(C * GW, dtype=np.int64)
    np.add.at(flat_cnt, ks_s * GW + g_s, 1)
    flat_start = np.concatenate([[0], np.cumsum(flat_cnt)])

    idx_slot = np.zeros((C, TOT), dtype=np.int16)
    w_slot = np.zeros((C, TOT), dtype=np.float32)
    win_slot = np.full((C, TOT), -1, dtype=np.int64)
    lane_slot = np.full((C, TOT), -1.0, dtype=np.float32)

    for sb in range(GSB):
        B = int(B_sb[sb])
        w1 = _win_sched(B)
        base = blk_of_sb[sb] * 128
        firsts = [int(np.where((w1 == v) | (w1 == v - 1))[0][0]) for v in range(SBW)]
        for c in range(C):
            pos = 0
            for v in range(SBW):
                gidx = sb * SBW + v
                n = cnt_cw[c, gidx]
                pos = max(pos, firsts[v] * 128)
                if n:
                    e0 = flat_start[c * GW + gidx]
                    sl = slice(base + pos, base + pos + n)
                    idx_slot[c, sl] = srow_s[e0:e0 + n]
                    w_slot[c, sl] = w_s[e0:e0 + n]
                    win_slot[c, sl] = gidx
                    lane_slot[c, sl] = lane_s[e0:e0 + n]
                    pos += n

    prim_w1 = np.empty(NBLK, dtype=np.int64)
    for sb in range(GSB):
        B = int(B_sb[sb])
        w1 = _win_sched(B)
        prim_w1[blk_of_sb[sb]:blk_of_sb[sb] + B] = sb * SBW + w1

    sec_needed = np.zeros(NBLK, dtype=bool)
    blk_of_slot = np.arange(TOT) // 128
    for c in range(C):
        m = win_slot[c] >= 0
        sec = win_slot[c][m] != prim_w1[blk_of_slot[m]]
        np.logical_or.at(sec_needed, blk_of_slot[m][sec], True)

    mm_block, mm_win = [], []
    for blk in range(NBLK):
        mm_block.append(blk); mm_win.append(int(prim_w1[blk]))
        if sec_needed[blk]:
            mm_block.append(blk); mm_win.append(int(prim_w1[blk]) + 1)
    mm_block = np.array(mm_block); mm_win = np.array(mm_win)
    NMM = len(mm_block)

    first_mm, last_mm = {}, {}
    for m in range(NMM):
        wn = int(mm_win[m])
        if wn not in first_mm:
            first_mm[wn] = m
        last_mm[wn] = m
    assert len(first_mm) == GW

    # per-block lane encoding vs the block's primary window:
    # lane + 128*(win - w1) for win in {w1, w1+1}, else -1
    ls = lane_slot.reshape(C, NBLK, 128)
    vs = win_slot.reshape(C, NBLK, 128)
    rel = vs - prim_w1[None, :, None]
    valid = (rel == 0) | (rel == 1)
    enc = np.where(valid, ls + 128.0 * rel, -1.0).astype(np.float32)
    dstl_blk = enc.transpose(0, 2, 1).copy()          # [C, 128, NBLK]
    wsl_blk = w_slot.reshape(C, NBLK, 128).transpose(0, 2, 1).copy()

    n_ops = (TOT + OPN - 1) // OPN
    op_sizes = [min(OPN, TOT - i * OPN) for i in range(n_ops)]
    idx_wrap = np.zeros((C, 16, TOT // 16), dtype=np.int16)
    off = 0
    for s in op_sizes:
        seg = idx_slot[:, off:off + s].reshape(C, s // 16, 16)
        idx_wrap[:, :, off // 16:(off + s) // 16] = seg.transpose(0, 2, 1)
        off += s
    idx_full = np.tile(idx_wrap, (1, 8, 1))

    o = np.arange(NODES_C)
    u_of = o // 128
    p_of = o % 128
    dinv_lane = np.ones((C, 128, W), dtype=np.float32)
    bl_lane = np.full((C, 128, W), 63.0, dtype=np.float32)
    gmin = np.zeros(C, dtype=np.int64)
    xT = np.zeros((C, IN_DIM, PADN), dtype=np.float32)
    for c in range(C):
        n0 = c * NODES_C
        dinv_lane[c, p_of, u_of] = dinv_full[n0:n0 + NODES_C]
        bseg = batch[n0:n0 + NODES_C]
        gmin[c] = bseg[0]
        assert int(bseg[-1] - bseg[0]) <= 62
        bl_lane[c, p_of, u_of] = (bseg - gmin[c]).astype(np.float32)
        xT[c, :, :NODES_C] = x[n0:n0 + NODES_C].T

    return dict(
        B_sb=B_sb, blk_of_sb=blk_of_sb, NBLK=NBLK, TOT=TOT, NMM=NMM,
        mm_block=mm_block, mm_win=mm_win, first_mm=first_mm, last_mm=last_mm,
        sec_needed=sec_needed, prim_w1=prim_w1,
        dstl_blk=dstl_blk, wsl_blk=wsl_blk, idx_full=idx_full,
        dinv_lane=dinv_lane, bl_lane=bl_lane, gmin=gmin, xT=xT,
        op_sizes=op_sizes,
    )


# --------------------------------------------------------------------------
def _build_program(prep, has_bias=True):
    import concourse.bacc as bacc
    import concourse.mybir as mybir
    import concourse.tile as tile
    from contextlib import ExitStack

    f32 = mybir.dt.float32
    bf16 = mybir.dt.bfloat16
    i16 = mybir.dt.int16
    OP = mybir.AluOpType
    AF = mybir.ActivationFunctionType

    NBLK = prep["NBLK"]
    TOT = prep["TOT"]
    NMM = prep["NMM"]
    mm_block = prep["mm_block"]
    mm_win = prep["mm_win"]
    first_mm = prep["first_mm"]
    last_mm = prep["last_mm"]
    op_sizes = prep["op_sizes"]
    blk_of_sb = prep["blk_of_sb"]

    # sb index of each global window; last window of each sb
    sb_of_win = np.arange(GW) // SBW

    nc = bacc.Bacc("TRN2", target_bir_lowering=False, debug=False, num_devices=C)

    xT_in = nc.declare_dram_parameter("xT", [IN_DIM, PADN], bf16, isOutput=False)
    W1_in = nc.declare_dram_parameter("W1", [IN_DIM, F], bf16, isOutput=False)
    W2_in = nc.declare_dram_parameter("W2", [F, F], bf16, isOutput=False)
    W3_in = nc.declare_dram_parameter("W3", [F, F], bf16, isOutput=False)
    ball_in = nc.declare_dram_parameter("ballw", [128, 3 * F], bf16, isOutput=False)
    iota256_in = nc.declare_dram_parameter("iota256", [128, 256], bf16, isOutput=False)
    iota64_in = nc.declare_dram_parameter("iota64", [128, F], bf16, isOutput=False)
    ident_in = nc.declare_dram_parameter("ident", [128, 128], bf16, isOutput=False)
    ones_in = nc.declare_dram_parameter("onescol", [128, 1], bf16, isOutput=False)
    dstl_in = nc.declare_dram_parameter("dstl", [128, NBLK], f32, isOutput=False)
    wsl_in = nc.declare_dram_parameter("wsl", [128, NBLK], f32, isOutput=False)
    idx_in = nc.declare_dram_parameter("idx16", [128, TOT // 16], i16, isOutput=False)
    dinv_in = nc.declare_dram_parameter("dinv", [128, W], f32, isOutput=False)
    dinvw_in = nc.declare_dram_parameter("dinvwide", [128, W * F], bf16, isOutput=False)
    bl_in = nc.declare_dram_parameter("batchloc", [128, W], f32, isOutput=False)
    pool_out = nc.declare_dram_parameter("pool_out", [F, F + 1], f32, isOutput=True)

    stk = ExitStack()
    tbl_sems = [stk.enter_context(nc.semaphore(f"tbl_{i}")) for i in range(3)]
    wr_sems = [stk.enter_context(nc.semaphore(f"wr_{i}")) for i in range(3)]
    cc_sems = [stk.enter_context(nc.semaphore(f"cc_{i}")) for i in range(3)]

    n_sb_dma = GSB // 2  # staging DMAs per layer (2 superblocks each)

    with tile.TileContext(nc, num_cores=C) as tc:
        tc.race_detector_enabled = False
        with (
            tc.tile_pool(name="persist", bufs=1) as pp,
            tc.tile_pool(name="idxp", bufs=3) as idxp,
            tc.tile_pool(name="msgp", bufs=3) as mp,
            tc.tile_pool(name="wstp", bufs=16) as wp,
            tc.tile_pool(name="stgp", bufs=3) as sgp,
            tc.tile_pool(name="epi", bufs=1) as ep,
            tc.tile_pool(name="ps", bufs=7, space="PSUM") as ps,
            tc.tile_pool(name="psPool", bufs=1, space="PSUM") as ps1,
            tc.tile_pool(name="dram", bufs=1, space="DRAM") as dr,
        ):
            def load(name, shape, dt, src):
                t = pp.tile(shape, dt, name=name)
                nc.sync.dma_start(out=t[:], in_=src[:])
                return t

            xT_a = load("xT_a", [IN_DIM, PADN], bf16, xT_in)
            w1 = load("w1", [IN_DIM, F], bf16, W1_in)
            w2 = load("w2", [F, F], bf16, W2_in)
            w3 = load("w3", [F, F], bf16, W3_in)
            ballw = load("ballw", [128, 3 * F], bf16, ball_in)
            iota256 = load("iota256", [128, 256], bf16, iota256_in)
            iota64 = load("iota64", [128, F], bf16, iota64_in)
            ident = load("ident", [128, 128], bf16, ident_in)
            onescol = load("onescol", [128, 1], bf16, ones_in)
            dstl = load("dstl", [128, NBLK], f32, dstl_in)
            wsl = load("wsl", [128, NBLK], f32, wsl_in)
            dinv = load("dinv", [128, W], f32, dinv_in)
            dinvw = load("dinvwide", [128, W * F], bf16, dinvw_in)
            batchloc = load("batchloc", [128, W], f32, bl_in)

            ytb = pp.tile([128, W * 128], bf16, name="ytb")
            xTn = pp.tile([F, PADN], bf16, name="xTn")
            acc = pp.tile([128, W * F], bf16, name="acc")
            xpr = pp.tile([128, W * F], bf16, name="xpr")
            rs_sb = pp.tile([128, W * F], bf16, name="rs_sb")

            for L in range(3):
                wmat = (w1, w2, w3)[L]

                # ---------------- phase A: y~ = dinv * (x @ W), dup bf16 ----
                for u in range(W):
                    psum_y = ps.tile([128, F], f32, name="psum_y", tag="psum_y",
                                     bufs=3)
                    lhsT = (xT_a if L == 0 else xTn)[:, u * 128:(u + 1) * 128]
                    nc.tensor.matmul(psum_y[:], lhsT, wmat[:], start=True, stop=True)
                    nc.vector.tensor_scalar(
                        ytb[:, u * 128:u * 128 + F], psum_y[:],
                        dinv[:, u:u + 1], None, OP.mult)
                    nc.scalar.activation(
                        ytb[:, u * 128 + F:(u + 1) * 128], psum_y[:],
                        AF.Copy, scale=dinv[:, u:u + 1])

                # table write (rows r = p*W+u, 256B each, contiguous per p)
                table = dr.tile([PADN, 128], bf16, name=f"table_{L}")
                nc.sync.dma_start(
                    out=table[:], in_=ytb[:].rearrange("p (u e) -> (p u) e", e=128)
                ).then_inc(tbl_sems[L], 16)

                partial = dr.tile([C * PADN, F], bf16, name=f"partial_{L}")
                rsout = dr.tile([PADN, F], bf16, name=f"rsout_{L}")

                nc.gpsimd.wait_ge(tbl_sems[L], 16)

                # ---------------- gathers + block matmuls -------------------
                n_ops = len(op_sizes)
                mts = [None] * n_ops
                op_base_blk = [0] * n_ops
                off = 0
                for k, s in enumerate(op_sizes):
                    op_base_blk[k] = off // 128
                    off += s

                size_regs = {}
                for s in set(op_sizes):
                    size_regs[s] = nc.gpsimd.to_reg(s)

                def issue_gather(k):
                    s = op_sizes[k]
                    off16 = sum(op_sizes[:k]) // 16
                    it = idxp.tile([128, OPN // 16], i16, name="idxt", tag="idxt")
                    nc.gpsimd.dma_start(out=it[:, :s // 16],
                                        in_=idx_in[:, off16:off16 + s // 16])
                    mt = mp.tile([128, OPN // 128, 128], bf16, name="msg", tag="msg")
                    nc.gpsimd.dma_gather(
                        out_ap=mt[:, :s // 128, :],
                        in_ap=table[:],
                        idxs_ap=it[:, :s // 16],
                        num_idxs=s,
                        num_idxs_reg=size_regs[s],
                        elem_size=128,
                    )
                    mts[k] = mt

                issue_gather(0)
                if n_ops > 1:
                    issue_gather(1)

                psums = {}
                stg = {}

                def _emit_window_close(wn):
                    sb2 = int(sb_of_win[wn]) // 2
                    v2 = wn % (2 * SBW)
                    if sb2 not in stg:
                        stg[sb2] = sgp.tile([128, 2 * SBW * F], bf16,
                                            name="stg", tag="stg")
                    nc.scalar.activation(
                        stg[sb2][:, v2 * F:(v2 + 1) * F], psums.pop(wn)[:],
                        AF.Copy)
                    if v2 == 2 * SBW - 1:
                        kk = wn // W
                        u0 = (wn % W) - (2 * SBW - 1)
                        dst_ap = partial[:].rearrange(
                            "(k p u) f -> k p (u f)", k=C, p=128
                        )[kk][:, u0 * F:(u0 + 2 * SBW) * F]
                        nc.sync.dma_start(
                            out=dst_ap, in_=stg.pop(sb2)[:]
                        ).then_inc(wr_sems[L], 16)

                m = 0
                cur_op = 0
                for blk in range(NBLK):
                    # advance gather op when first matmul touches its blocks
                    while cur_op + 1 < n_ops and blk >= op_base_blk[cur_op + 1]:
                        cur_op += 1
                    for ahead in (1, 2):
                        if cur_op + ahead < n_ops and mts[cur_op + ahead] is None:
                            issue_gather(cur_op + ahead)
                    mt = mts[cur_op]
                    jloc = blk - op_base_blk[cur_op]

                    sec = bool(prep["sec_needed"][blk])
                    width = 256 if sec else 128
                    wst = wp.tile([128, 256], bf16, name="wst", tag="wst")
                    nc.vector.tensor_scalar(
                        wst[:, :width], iota256[:, :width],
                        dstl[:, blk:blk + 1], wsl[:, blk:blk + 1],
                        OP.is_equal, OP.mult)

                    for half in range(2 if sec else 1):
                        wn = int(mm_win[m])
                        if wn not in psums:
                            psums[wn] = ps.tile([128, F], f32, name="psum_w",
                                                tag="psum_w", bufs=3)
                        nc.tensor.matmul(
                            psums[wn][:], wst[:, half * 128:(half + 1) * 128],
                            mt[:, jloc, 0:F],
                            start=(m == first_mm[wn]), stop=(m == last_mm[wn]),
                            skip_group_check=True)
                        m += 1
                        if (m - 1) == last_mm[wn]:
                            _emit_window_close(wn)
                assert m == NMM

                # ---------------- ReduceScatter -----------------------------
                nc.gpsimd.wait_ge(wr_sems[L], 16 * n_sb_dma)
                nc.gpsimd.collective_compute(
                    "ReduceScatter",
                    OP.add,
                    replica_groups=[list(range(C))],
                    ins=[partial[:]],
                    outs=[rsout[:]],
                ).then_inc(cc_sems[L], 1)
                nc.gpsimd.wait_ge(cc_sems[L], 1)
                nc.gpsimd.dma_start(
                    out=rs_sb[:],
                    in_=rsout[:].rearrange("(p u) f -> p (u f)", p=128))

                # ---------------- wide epilogue -----------------------------
                ytb_self = ytb[:].rearrange("p (u e) -> p u e", e=128)[:, :, 0:F]
                nc.vector.tensor_tensor(xpr[:], rs_sb[:], ytb_self, OP.add)
                nc.vector.tensor_tensor(xpr[:], xpr[:], dinvw[:], OP.mult)
                if has_bias:
                    for u in range(W):
                        nc.vector.tensor_tensor(
                            xpr[:, u * F:(u + 1) * F], xpr[:, u * F:(u + 1) * F],
                            ballw[:, L * F:(L + 1) * F], OP.add)
                nc.vector.tensor_scalar(xpr[:], xpr[:], 0.0, None, OP.max)
                if L == 0:
                    nc.vector.tensor_copy(acc[:], xpr[:])
                else:
                    nc.vector.tensor_tensor(acc[:], acc[:], xpr[:], OP.add)

                if L < 2:
                    for u in range(W):
                        ptr = ps.tile([F, 128], bf16, name="ptr", tag="ptr",
                                      bufs=1)
                        nc.tensor.transpose(ptr[:], xpr[:, u * F:(u + 1) * F],
                                            ident[:])
                        eng = nc.gpsimd if u % 2 == 0 else nc.vector
                        eng.tensor_copy(xTn[:, u * 128:(u + 1) * 128], ptr[:])

            # ---------------- pooling -----------------------------------
            psum_pool = ps1.tile([F, F + 1], f32, name="psum_pool")
            psum_sums = psum_pool[:, 0:F]
            psum_cnt = psum_pool[:, F:F + 1]
            for u in range(W):
                sg = wp.tile([128, F], bf16, name="sg", tag="sg")
                nc.vector.tensor_scalar(
                    sg[:], iota64[:], batchloc[:, u:u + 1], None, OP.is_equal)
                nc.tensor.matmul(
                    psum_sums, sg[:], acc[:, u * F:(u + 1) * F],
                    start=(u == 0), stop=(u == W - 1), skip_group_check=True)
                nc.tensor.matmul(
                    psum_cnt, sg[:], onescol[:],
                    start=(u == 0), stop=(u == W - 1), skip_group_check=True)
            outt = ep.tile([F, F + 1], f32, name="outt")
            nc.vector.tensor_copy(outt[:, :F], psum_sums)
            nc.vector.tensor_copy(outt[:, F:F + 1], psum_cnt)
            nc.sync.dma_start(out=pool_out[:], in_=outt[:])

    stk.close()
    nc.compile()
    _split_waits(nc)
    return nc


# --------------------------------------------------------------------------
def kernel(x, edge_weight, W1, b1, W2, b2, W3, b3, Wl, bl, edge_index, batch):
    from concourse.bass_utils import run_bass_kernel_spmd
    import jax.numpy as jnp

    prep = _host_prep(x, edge_weight, edge_index, batch)

    has_bias = any(np.any(np.asarray(b) != 0) for b in (b1, b2, b3))
    cache_key = (prep["NBLK"], prep["NMM"], tuple(prep["op_sizes"][:3]), has_bias)
    if cache_key not in _prog_cache:
        _prog_cache[cache_key] = _build_program(prep, has_bias=has_bias)
    nc = _prog_cache[cache_key]

    bf = lambda a: np.asarray(jnp.asarray(np.asarray(a, np.float32), jnp.bfloat16))
    W1b, W2b, W3b = bf(W1), bf(W2), bf(W3)
    ballw = np.zeros((128, 3 * F), dtype=np.float32)
    ballw[:, 0:F] = np.asarray(b1, np.float32)[None, :]
    ballw[:, F:2 * F] = np.asarray(b2, np.float32)[None, :]
    ballw[:, 2 * F:3 * F] = np.asarray(b3, np.float32)[None, :]
    ballw = bf(ballw)
    iota256 = bf(np.tile(np.arange(256, dtype=np.float32)[None, :], (128, 1)))
    iota64 = bf(np.tile(np.arange(F, dtype=np.float32)[None, :], (128, 1)))
    ident = bf(np.eye(128, dtype=np.float32))
    onescol = bf(np.ones((128, 1), dtype=np.float32))
    xT_bf = bf(prep["xT"])
    dinvw = bf(np.repeat(prep["dinv_lane"], F, axis=2))  # [C,128,W*F]

    in_maps = []
    for c in range(C):
        in_maps.append({
            "xT": xT_bf[c],
            "W1": W1b, "W2": W2b, "W3": W3b, "ballw": ballw,
            "iota256": iota256, "iota64": iota64, "ident": ident,
            "onescol": onescol,
            "dstl": prep["dstl_blk"][c], "wsl": prep["wsl_blk"][c],
            "idx16": prep["idx_full"][c],
            "dinv": prep["dinv_lane"][c], "dinvwide": dinvw[c],
            "batchloc": prep["bl_lane"][c],
        })

    res = run_bass_kernel_spmd(nc, in_maps, core_ids=list(range(C)))

    sums = np.zeros((N_GRAPHS, F), dtype=np.float64)
    cnts = np.zeros(N_GRAPHS, dtype=np.float64)
    for c in range(C):
        out = res.results[c]["pool_out"]
        g0 = int(prep["gmin"][c])
        for r in range(63):
            g = g0 + r
            if g < N_GRAPHS:
                sums[g] += out[r, :F]
                cnts[g] += out[r, F]
    pooled = (sums / 3.0) / np.maximum(cnts, 1.0)[:, None]
    logits = pooled @ np.asarray(Wl, np.float64) + np.asarray(bl, np.float64)
    z = logits - logits.max(axis=1, keepdims=True)
    ez = np.exp(z)
    return (ez / ez.sum(axis=1, keepdims=True)).astype(np.float32)


# revision 43
# speedup vs baseline: 1.4538x; 1.0349x over previous
"""GCN (3-layer + mean-pool + linear + softmax) on 8 Trainium2 NeuronCores.

Push-mode graph parallelism: each core owns a contiguous 12,500-node range
(padded 12544 = 128 lanes x 98 windows). Per layer, each core:
  phase A:  y~ = dinv * (x @ W) on PE, written as a bf16 table with duplicated
            rows [y~|y~] (256B rows) to DRAM,
  gathers   its OWN out-edge sources from the local table (int16 row ids,
            single chunk, large batched dma_gather ops),
  scatters  messages into per-global-window PSUM accumulators via fused
            one-hot matmuls: lhsT = (iota==dstlane)*|w| built in one
            tensor_scalar(is_equal,mult) op (DVE 4x mode / gpsimd),
  writes    bf16 partial sums [100352, 64] (partition-major rows -> large
            contiguous DMA descriptors),
  ReduceScatter (out 12544x64 bf16 ~= 55us) delivers summed aggregates for its
            own nodes; wide fused epilogue: x' = relu(dinv*(rs + y~self) + b).
Static SPMD schedule: superblocks of 7 windows with ~15 blocks each; blocks
serve a primary window and optionally the next (straddle), absorbing per-core
count variation with <=7% slot padding. Pooling via one-hot(graph) matmuls;
host applies the final 64x10 linear + softmax.
"""
import os
import sys
import numpy as np

sys.path.insert(0, os.path.dirname(os.path.abspath(__file__)))

N_NODES = 100000
N_GRAPHS = 256
IN_DIM = 128
F = 64
C = 8
NODES_C = 12500
PADN = 12544
W = 98
SBW = 7
NSB = W // SBW
GW = C * W
GSB = C * NSB
OPN = 8192
POOL_BUILD_FRAC = 0.0    # fraction of one-hot builds on gpsimd (Pool)

_prog_cache = {}


def _win_sched(B):
    return np.minimum((np.arange(B) * SBW) // B, SBW - 1)


# --------------------------------------------------------------------------
def _split_waits(nc, cap=1):
    """Walrus rejects >1 sem wait per instruction; hoist extras onto injected
    same-engine InstEventSemaphore waits."""
    import concourse.mybir as mybir
    uid = [0]
    n_fixed = 0
    for fn in nc.m.functions:
        for bb in fn.blocks:
            insts = bb.instructions
            new_list = []
            for inst in insts:
                si = inst.sync_info
                waits = list(si.on_wait) if si and si.on_wait else []
                if len(waits) > cap:
                    extra, keep = waits[:-cap], waits[-cap:]
                    for wv in extra:
                        uid[0] += 1
                        nop = mybir.InstEventSemaphore(name=f"waitfix_{uid[0]}")
                        nop.engine = inst.engine
                        nop.sync_info = mybir.SyncInfo(on_wait=[wv], on_update=[])
                        new_list.append(nop)
                    si.on_wait = keep
                    n_fixed += 1
                new_list.append(inst)
            if len(new_list) != len(insts):
                try:
                    bb.instructions = new_list
                except Exception:
                    insts.clear()
                    insts.extend(new_list)
    return n_fixed


# --------------------------------------------------------------------------
def _host_prep(x, edge_weight, edge_index, batch):
    src = np.asarray(edge_index[0], dtype=np.int64)
    dst = np.asarray(edge_index[1], dtype=np.int64)
    w_abs = np.abs(np.asarray(edge_weight, dtype=np.float32))
    batch = np.asarray(batch, dtype=np.int64)
    x = np.asarray(x, dtype=np.float32)

    deg = np.bincount(dst, weights=w_abs.astype(np.float64), minlength=N_NODES) + 1.0
    dinv_full = (1.0 / np.sqrt(deg)).astype(np.float32)

    ks = src // NODES_C
    so = src - ks * NODES_C
    srow = ((so % 128) * W + so // 128).astype(np.int16)
    kd = dst // NODES_C
    do = dst - kd * NODES_C
    lane = (do % 128).astype(np.float32)
    g = kd * W + do // 128

    # superblock processing order: A-half (u<49) sbs of all cores first
    order_sb = sorted(range(GSB), key=lambda s: ((s % (2 * SBW)) >= SBW, s))
    rank_of = np.empty(GSB, dtype=np.int64)
    for r, s in enumerate(order_sb):
        rank_of[s] = r

    gsb = g // SBW
    key = (ks * GSB + rank_of[gsb]) * SBW + (g % SBW)
    order = np.argsort(key, kind="stable")
    ks_s, g_s = ks[order], g[order]
    srow_s, lane_s, w_s = srow[order], lane[order], w_abs[order]

    cnt_cw = np.zeros((C, GW), dtype=np.int64)
    np.add.at(cnt_cw, (ks_s, g_s), 1)
    cnt_csb = cnt_cw.reshape(C, GSB, SBW).sum(axis=2)

    B_sb = np.maximum(np.ceil(cnt_csb.max(axis=0) / 128).astype(np.int64), SBW)

    def feasible(sb, B):
        w1 = _win_sched(B)
        firsts = [np.where((w1 == v) | (w1 == v - 1))[0][0] for v in range(SBW)]
        lasts = [np.where(w1 == v)[0][-1] for v in range(SBW)]
        for c in range(C):
            pos = 0
            for v in range(SBW):
                pos = max(pos, int(firsts[v]) * 128)
                pos += cnt_cw[c, sb * SBW + v]
                if pos > (int(lasts[v]) + 1) * 128:
                    return False
        return True

    for sb in range(GSB):
        while not feasible(sb, int(B_sb[sb])):
            B_sb[sb] += 1

    B_rank = np.array([B_sb[s] for s in order_sb])
    blk_of_rank = np.concatenate([[0], np.cumsum(B_rank)])
    NBLK = int(blk_of_rank[-1])
    TOT = NBLK * 128
    nA_blk = int(blk_of_rank[GSB // 2])   # blocks in the A half

    flat_cnt = np.zeros(C * GW, dtype=np.int64)
    np.add.at(flat_cnt, ks_s * GW + g_s, 1)
    flat_start = np.concatenate([[0], np.cumsum(flat_cnt)])

    idx_slot = np.zeros((C, TOT), dtype=np.int16)
    w_slot = np.zeros((C, TOT), dtype=np.float32)
    win_slot = np.full((C, TOT), -1, dtype=np.int64)
    lane_slot = np.full((C, TOT), -1.0, dtype=np.float32)

    for r in range(GSB):
        sb = order_sb[r]
        B = int(B_sb[sb])
        w1 = _win_sched(B)
        base = blk_of_rank[r] * 128
        firsts = [int(np.where((w1 == v) | (w1 == v - 1))[0][0]) for v in range(SBW)]
        for c in range(C):
            pos = 0
            for v in range(SBW):
                gidx = sb * SBW + v
                n = cnt_cw[c, gidx]
                pos = max(pos, firsts[v] * 128)
                if n:
                    e0 = flat_start[c * GW + gidx]
                    sl = slice(base + pos, base + pos + n)
                    idx_slot[c, sl] = srow_s[e0:e0 + n]
                    w_slot[c, sl] = w_s[e0:e0 + n]
                    win_slot[c, sl] = gidx
                    lane_slot[c, sl] = lane_s[e0:e0 + n]
                    pos += n

    prim_w1 = np.empty(NBLK, dtype=np.int64)
    for r in range(GSB):
        sb = order_sb[r]
        B = int(B_sb[sb])
        w1 = _win_sched(B)
        prim_w1[blk_of_rank[r]:blk_of_rank[r] + B] = sb * SBW + w1

    sec_needed = np.zeros(NBLK, dtype=bool)
    blk_of_slot = np.arange(TOT) // 128
    for c in range(C):
        m = win_slot[c] >= 0
        sec = win_slot[c][m] != prim_w1[blk_of_slot[m]]
        np.logical_or.at(sec_needed, blk_of_slot[m][sec], True)

    mm_block, mm_win = [], []
    for blk in range(NBLK):
        mm_block.append(blk); mm_win.append(int(prim_w1[blk]))
        if sec_needed[blk]:
            mm_block.append(blk); mm_win.append(int(prim_w1[blk]) + 1)
    mm_block = np.array(mm_block); mm_win = np.array(mm_win)
    NMM = len(mm_block)

    first_mm, last_mm = {}, {}
    for m in range(NMM):
        wn = int(mm_win[m])
        if wn not in first_mm:
            first_mm[wn] = m
        last_mm[wn] = m
    assert len(first_mm) == GW

    # per-block lane encoding vs the block's primary window:
    # lane + 128*(win - w1) for win in {w1, w1+1}, else -1
    ls = lane_slot.reshape(C, NBLK, 128)
    vs = win_slot.reshape(C, NBLK, 128)
    rel = vs - prim_w1[None, :, None]
    valid = (rel == 0) | (rel == 1)
    enc = np.where(valid, ls + 128.0 * rel, -1.0).astype(np.float32)
    dstl_blk = enc.transpose(0, 2, 1).copy()          # [C, 128, NBLK]
    wsl_blk = w_slot.reshape(C, NBLK, 128).transpose(0, 2, 1).copy()

    n_ops = (TOT + OPN - 1) // OPN
    op_sizes = [min(OPN, TOT - i * OPN) for i in range(n_ops)]
    idx_wrap = np.zeros((C, 16, TOT // 16), dtype=np.int16)
    off = 0
    for s in op_sizes:
        seg = idx_slot[:, off:off + s].reshape(C, s // 16, 16)
        idx_wrap[:, :, off // 16:(off + s) // 16] = seg.transpose(0, 2, 1)
        off += s
    idx_full = np.tile(idx_wrap, (1, 8, 1))

    o = np.arange(NODES_C)
    u_of = o // 128
    p_of = o % 128
    dinv_lane = np.ones((C, 128, W), dtype=np.float32)
    bl_lane = np.full((C, 128, W), 63.0, dtype=np.float32)
    gmin = np.zeros(C, dtype=np.int64)
    xT = np.zeros((C, IN_DIM, PADN), dtype=np.float32)
    for c in range(C):
        n0 = c * NODES_C
        dinv_lane[c, p_of, u_of] = dinv_full[n0:n0 + NODES_C]
        bseg = batch[n0:n0 + NODES_C]
        gmin[c] = bseg[0]
        assert int(bseg[-1] - bseg[0]) <= 62
        bl_lane[c, p_of, u_of] = (bseg - gmin[c]).astype(np.float32)
        xT[c, :, :NODES_C] = x[n0:n0 + NODES_C].T

    return dict(
        B_sb=B_sb, NBLK=NBLK, TOT=TOT, NMM=NMM, nA_blk=nA_blk,
        mm_block=mm_block, mm_win=mm_win, first_mm=first_mm, last_mm=last_mm,
        sec_needed=sec_needed, prim_w1=prim_w1,
        dstl_blk=dstl_blk, wsl_blk=wsl_blk, idx_full=idx_full,
        dinv_lane=dinv_lane, bl_lane=bl_lane, gmin=gmin, xT=xT,
        op_sizes=op_sizes,
        idx_slot=idx_slot, w_slot=w_slot, win_slot=win_slot,
        lane_slot=lane_slot,
    )


# --------------------------------------------------------------------------
def _build_program(prep, has_bias=True):
    import concourse.bacc as bacc
    import concourse.mybir as mybir
    import concourse.tile as tile
    from contextlib import ExitStack

    f32 = mybir.dt.float32
    bf16 = mybir.dt.bfloat16
    i16 = mybir.dt.int16
    OP = mybir.AluOpType
    AF = mybir.ActivationFunctionType

    NBLK = prep["NBLK"]
    TOT = prep["TOT"]
    NMM = prep["NMM"]
    mm_block = prep["mm_block"]
    mm_win = prep["mm_win"]
    first_mm = prep["first_mm"]
    last_mm = prep["last_mm"]
    op_sizes = prep["op_sizes"]

    nc = bacc.Bacc("TRN2", target_bir_lowering=False, debug=False, num_devices=C)

    xT_in = nc.declare_dram_parameter("xT", [IN_DIM, PADN], bf16, isOutput=False)
    W1_in = nc.declare_dram_parameter("W1", [IN_DIM, F], bf16, isOutput=False)
    W2_in = nc.declare_dram_parameter("W2", [F, F], bf16, isOutput=False)
    W3_in = nc.declare_dram_parameter("W3", [F, F], bf16, isOutput=False)
    ball_in = nc.declare_dram_parameter("ballw", [128, 3 * F], bf16, isOutput=False)
    iota256_in = nc.declare_dram_parameter("iota256", [128, 256], bf16, isOutput=False)
    iota64_in = nc.declare_dram_parameter("iota64", [128, F], bf16, isOutput=False)
    ident_in = nc.declare_dram_parameter("ident", [128, 128], bf16, isOutput=False)
    ones_in = nc.declare_dram_parameter("onescol", [128, 1], bf16, isOutput=False)
    dstl_in = nc.declare_dram_parameter("dstl", [128, NBLK], f32, isOutput=False)
    wsl_in = nc.declare_dram_parameter("wsl", [128, NBLK], f32, isOutput=False)
    idx_in = nc.declare_dram_parameter("idx16", [128, TOT // 16], i16, isOutput=False)
    dinv_in = nc.declare_dram_parameter("dinv", [128, W], f32, isOutput=False)
    dinvw_in = nc.declare_dram_parameter("dinvwide", [128, W * F], bf16, isOutput=False)
    bl_in = nc.declare_dram_parameter("batchloc", [128, W], f32, isOutput=False)
    pool_out = nc.declare_dram_parameter("pool_out", [F, F + 1], f32, isOutput=True)

    stk = ExitStack()
    tbl_sems = [stk.enter_context(nc.semaphore(f"tbl_{i}")) for i in range(3)]
    wrA_sems = [stk.enter_context(nc.semaphore(f"wrA_{i}")) for i in range(3)]
    wrB_sems = [stk.enter_context(nc.semaphore(f"wrB_{i}")) for i in range(3)]
    ccA_sems = [stk.enter_context(nc.semaphore(f"ccA_{i}")) for i in range(3)]
    ccB_sems = [stk.enter_context(nc.semaphore(f"ccB_{i}")) for i in range(3)]

    HW = W // 2 + (W % 2 > 0)  # 49 windows per half
    HN = PADN // 2             # 6272 nodes per half
    # staging groups within a half: u-offsets and sizes
    GRP0 = [0, 14, 28, 42]
    GRPS = [14, 14, 14, 7]
    N_WR = len(GRP0) * C       # staging DMAs per half per layer

    with tile.TileContext(nc, num_cores=C) as tc:
        tc.race_detector_enabled = False
        with (
            tc.tile_pool(name="persist", bufs=1) as pp,
            tc.tile_pool(name="idxp", bufs=3) as idxp,
            tc.tile_pool(name="msgp", bufs=3) as mp,
            tc.tile_pool(name="wstp", bufs=16) as wp,
            tc.tile_pool(name="stgp", bufs=3) as sgp,
            tc.tile_pool(name="epi", bufs=1) as ep,
            tc.tile_pool(name="ps", bufs=7, space="PSUM") as ps,
            tc.tile_pool(name="psPool", bufs=1, space="PSUM") as ps1,
            tc.tile_pool(name="dram", bufs=1, space="DRAM") as dr,
        ):
            def load(name, shape, dt, src):
                t = pp.tile(shape, dt, name=name)
                nc.sync.dma_start(out=t[:], in_=src[:])
                return t

            xT_a = load("xT_a", [IN_DIM, PADN], bf16, xT_in)
            w1 = load("w1", [IN_DIM, F], bf16, W1_in)
            w2 = load("w2", [F, F], bf16, W2_in)
            w3 = load("w3", [F, F], bf16, W3_in)
            ballw = load("ballw", [128, 3 * F], bf16, ball_in)
            iota256 = load("iota256", [128, 256], bf16, iota256_in)
            iota64 = load("iota64", [128, F], bf16, iota64_in)
            ident = load("ident", [128, 128], bf16, ident_in)
            onescol = load("onescol", [128, 1], bf16, ones_in)
            dstl = load("dstl", [128, NBLK], f32, dstl_in)
            wsl = load("wsl", [128, NBLK], f32, wsl_in)
            dinv = load("dinv", [128, W], f32, dinv_in)
            dinvw = load("dinvwide", [128, W * F], bf16, dinvw_in)
            batchloc = load("batchloc", [128, W], f32, bl_in)

            ytb = pp.tile([128, W * 128], bf16, name="ytb")
            xTn = pp.tile([F, PADN], bf16, name="xTn")
            acc = pp.tile([128, W * F], bf16, name="acc")
            xpr = pp.tile([128, W * F], bf16, name="xpr")
            rs_sbA = pp.tile([128, HW * F], bf16, name="rs_sbA")
            rs_sbB = pp.tile([128, (W - HW) * F], bf16, name="rs_sbB")

            psum_pool = ps1.tile([F, F + 1], f32, name="psum_pool")
            psum_sums = psum_pool[:, 0:F]
            psum_cnt = psum_pool[:, F:F + 1]
            pool_mm = [0]  # matmul counter for start flags

            tables = [dr.tile([PADN, 128], bf16, name=f"table_{L}")
                      for L in range(3)]
            partsA = [dr.tile([C * HN, F], bf16, name=f"partA_{L}")
                      for L in range(3)]
            partsB = [dr.tile([C * (PADN - HN), F], bf16, name=f"partB_{L}")
                      for L in range(3)]
            rsoutA = [dr.tile([HN, F], bf16, name=f"rsoutA_{L}")
                      for L in range(3)]
            rsoutB = [dr.tile([PADN - HN, F], bf16, name=f"rsoutB_{L}")
                      for L in range(3)]

            def phase_a(L, u0, u1):
                """y~ = dinv*(x@W) for windows [u0,u1); dup bf16 into ytb;
                then write the table rows for that half."""
                wmat = (w1, w2, w3)[L]
                for u in range(u0, u1):
                    psum_y = ps.tile([128, F], f32, name="psum_y", tag="psum_y",
                                     bufs=2)
                    lhsT = (xT_a if L == 0 else xTn)[:, u * 128:(u + 1) * 128]
                    nc.tensor.matmul(psum_y[:], lhsT, wmat[:], start=True,
                                     stop=True)
                    nc.vector.tensor_scalar(
                        ytb[:, u * 128:u * 128 + F], psum_y[:],
                        dinv[:, u:u + 1], None, OP.mult)
                    nc.scalar.activation(
                        ytb[:, u * 128 + F:(u + 1) * 128], psum_y[:],
                        AF.Copy, scale=dinv[:, u:u + 1])
                tbl_ap = tables[L][:].rearrange("(p u) e -> p u e", p=128)
                nc.sync.dma_start(
                    out=tbl_ap[:, u0:u1, :],
                    in_=ytb[:, u0 * 128:u1 * 128].rearrange(
                        "p (u e) -> p u e", e=128),
                ).then_inc(tbl_sems[L], 16)

            def epilogue(L, half):
                """x' = relu(dinv*(rs + y~self) + b) for one half; acc/pool."""
                u0 = 0 if half == 0 else HW
                u1 = HW if half == 0 else W
                nw = u1 - u0
                rs_sb = rs_sbA if half == 0 else rs_sbB
                rso = (rsoutA if half == 0 else rsoutB)[L]
                cc = (ccA_sems if half == 0 else ccB_sems)[L]
                nc.sync.wait_ge(cc, 1)
                nc.sync.dma_start(
                    out=rs_sb[:],
                    in_=rso[:].rearrange("(p u) f -> p (u f)", p=128))
                xs = xpr[:, u0 * F:u1 * F]
                ytb_self = ytb[:].rearrange(
                    "p (u e) -> p u e", e=128)[:, u0:u1, 0:F]
                nc.vector.tensor_tensor(xs, rs_sb[:], ytb_self, OP.add)
                nc.vector.tensor_tensor(xs, xs, dinvw[:, u0 * F:u1 * F],
                                        OP.mult)
                if has_bias:
                    for u in range(u0, u1):
                        nc.vector.tensor_tensor(
                            xpr[:, u * F:(u + 1) * F],
                            xpr[:, u * F:(u + 1) * F],
                            ballw[:, L * F:(L + 1) * F], OP.add)
                nc.vector.tensor_scalar(xs, xs, 0.0, None, OP.max)
                if L == 0:
                    nc.vector.tensor_copy(acc[:, u0 * F:u1 * F], xs)
                elif L == 1:
                    nc.vector.tensor_tensor(acc[:, u0 * F:u1 * F],
                                            acc[:, u0 * F:u1 * F], xs, OP.add)
                if L < 2:
                    for u in range(u0, u1):
                        ptr = ps.tile([F, 128], bf16, name="ptr", tag="ptr",
                                      bufs=2)
                        nc.tensor.transpose(ptr[:], xpr[:, u * F:(u + 1) * F],
                                            ident[:])
                        eng = nc.gpsimd if u % 2 == 0 else nc.vector
                        eng.tensor_copy(xTn[:, u * 128:(u + 1) * 128], ptr[:])
                    phase_a(L + 1, u0, u1)
                else:
                    pool_pass(xpr, u0, u1, last=(half == 1))

            def pool_pass(src_tile, u0, u1, last):
                for u in range(u0, u1):
                    sg = wp.tile([128, 256], bf16, name="sg", tag="wst")
                    nc.vector.tensor_scalar(
                        sg[:, :F], iota64[:], batchloc[:, u:u + 1], None,
                        OP.is_equal)
                    first = pool_mm[0] == 0
                    stop = last and (u == u1 - 1)
                    nc.tensor.matmul(
                        psum_sums, sg[:, :F], src_tile[:, u * F:(u + 1) * F],
                        start=first, stop=stop, skip_group_check=True)
                    nc.tensor.matmul(
                        psum_cnt, sg[:, :F], onescol[:],
                        start=first, stop=stop, skip_group_check=True)
                    pool_mm[0] += 1

            # ---------------- initial phase A (layer 0) ---------------------
            phase_a(0, 0, HW)
            phase_a(0, HW, W)

            for L in range(3):
                partial_h = (partsA[L], partsB[L])
                rsout_h = (rsoutA[L], rsoutB[L])
                wr_h = (wrA_sems[L], wrB_sems[L])
                cc_h = (ccA_sems[L], ccB_sems[L])

                nc.gpsimd.wait_ge(tbl_sems[L], 32)

                # pooling pass 1 over acc = x1+x2 runs during layer-2 blocks
                if L == 2:
                    pool_pass(acc, 0, W, last=False)

                n_ops = len(op_sizes)
                mts = [None] * n_ops
                op_base_blk = [0] * n_ops
                off = 0
                for k, s in enumerate(op_sizes):
                    op_base_blk[k] = off // 128
                    off += s

                size_regs = {}
                for s in set(op_sizes):
                    size_regs[s] = nc.gpsimd.to_reg(s)

                def issue_gather(k):
                    s = op_sizes[k]
                    off16 = sum(op_sizes[:k]) // 16
                    it = idxp.tile([128, OPN // 16], i16, name="idxt",
                                   tag="idxt")
                    nc.gpsimd.dma_start(out=it[:, :s // 16],
                                        in_=idx_in[:, off16:off16 + s // 16])
                    mt = mp.tile([128, OPN // 128, 128], bf16, name="msg",
                                 tag="msg")
                    nc.gpsimd.dma_gather(
                        out_ap=mt[:, :s // 128, :],
                        in_ap=tables[L][:],
                        idxs_ap=it[:, :s // 16],
                        num_idxs=s,
                        num_idxs_reg=size_regs[s],
                        elem_size=128,
                    )
                    mts[k] = mt

                issue_gather(0)
                if n_ops > 1:
                    issue_gather(1)

                psums = {}
                stg = {}

                def window_close(wn):
                    u = wn % W
                    kk = wn // W
                    half = 0 if u < HW else 1
                    uu = u - (0 if half == 0 else HW)
                    grp = min(uu // 14, len(GRP0) - 1)
                    gkey = (kk, half, grp)
                    sz = GRPS[grp]
                    if gkey not in stg:
                        stg[gkey] = sgp.tile([128, 14 * F], bf16, name="stg",
                                             tag="stg")
                    voff = uu - GRP0[grp]
                    nc.scalar.activation(
                        stg[gkey][:, voff * F:(voff + 1) * F],
                        psums.pop(wn)[:], AF.Copy)
                    if voff == sz - 1:
                        part = partial_h[half]
                        nh = HW if half == 0 else W - HW
                        dst_ap = part[:].rearrange(
                            "(k p u) f -> k p (u f)", k=C, p=128
                        )[kk][:, GRP0[grp] * F:(GRP0[grp] + sz) * F]
                        nc.sync.dma_start(
                            out=dst_ap, in_=stg.pop(gkey)[:, :sz * F]
                        ).then_inc(wr_h[half], 16)

                nA_blk = prep["nA_blk"]
                trigger_blk = nA_blk + int(0.35 * (NBLK - nA_blk))
                closedA = [0]

                m = 0
                cur_op = 0
                for blk in range(NBLK):
                    while cur_op + 1 < n_ops and blk >= op_base_blk[cur_op + 1]:
                        cur_op += 1
                    for ahead in (1, 2):
                        if cur_op + ahead < n_ops and mts[cur_op + ahead] is None:
                            issue_gather(cur_op + ahead)
                    mt = mts[cur_op]
                    jloc = blk - op_base_blk[cur_op]

                    sec = bool(prep["sec_needed"][blk])
                    width = 256 if sec else 128
                    wst = wp.tile([128, 256], bf16, name="wst", tag="wst")
                    nc.vector.tensor_scalar(
                        wst[:, :width], iota256[:, :width],
                        dstl[:, blk:blk + 1], wsl[:, blk:blk + 1],
                        OP.is_equal, OP.mult)

                    for half_mm in range(2 if sec else 1):
                        wn = int(mm_win[m])
                        if wn not in psums:
                            psums[wn] = ps.tile([128, F], f32, name="psum_w",
                                                tag="psum_w", bufs=3)
                        nc.tensor.matmul(
                            psums[wn][:],
                            wst[:, half_mm * 128:(half_mm + 1) * 128],
                            mt[:, jloc, 0:F],
                            start=(m == first_mm[wn]), stop=(m == last_mm[wn]),
                            skip_group_check=True)
                        m += 1
                        if (m - 1) == last_mm[wn]:
                            window_close(wn)
                            if (wn % W) < HW:
                                closedA[0] += 1
                                if closedA[0] == HW * C:
                                    # all A windows staged: kick RS_A
                                    nc.gpsimd.wait_ge(wr_h[0], 16 * N_WR)
                                    nc.gpsimd.collective_compute(
                                        "ReduceScatter", OP.add,
                                        replica_groups=[list(range(C))],
                                        ins=[partial_h[0][:]],
                                        outs=[rsout_h[0][:]],
                                    ).then_inc(cc_h[0], 1)

                    if blk == trigger_blk:
                        epilogue(L, 0)

                assert m == NMM

                # RS_B + exposed B boundary
                nc.gpsimd.wait_ge(wr_h[1], 16 * N_WR)
                nc.gpsimd.collective_compute(
                    "ReduceScatter", OP.add,
                    replica_groups=[list(range(C))],
                    ins=[partial_h[1][:]],
                    outs=[rsout_h[1][:]],
                ).then_inc(cc_h[1], 1)
                epilogue(L, 1)

            # ---------------- pooling output ---------------------------------
            outt = ep.tile([F, F + 1], f32, name="outt")
            nc.vector.tensor_copy(outt[:, :F], psum_sums)
            nc.vector.tensor_copy(outt[:, F:F + 1], psum_cnt)
            nc.sync.dma_start(out=pool_out[:], in_=outt[:])

    stk.close()
    nc.compile()
    _split_waits(nc)
    return nc


# --------------------------------------------------------------------------
def kernel(x, edge_weight, W1, b1, W2, b2, W3, b3, Wl, bl, edge_index, batch):
    from concourse.bass_utils import run_bass_kernel_spmd
    import jax.numpy as jnp

    prep = _host_prep(x, edge_weight, edge_index, batch)

    has_bias = any(np.any(np.asarray(b) != 0) for b in (b1, b2, b3))
    cache_key = (prep["NBLK"], prep["NMM"], tuple(prep["op_sizes"][:3]), has_bias)
    if cache_key not in _prog_cache:
        _prog_cache[cache_key] = _build_program(prep, has_bias=has_bias)
    nc = _prog_cache[cache_key]

    bf = lambda a: np.asarray(jnp.asarray(np.asarray(a, np.float32), jnp.bfloat16))
    W1b, W2b, W3b = bf(W1), bf(W2), bf(W3)
    ballw = np.zeros((128, 3 * F), dtype=np.float32)
    ballw[:, 0:F] = np.asarray(b1, np.float32)[None, :]
    ballw[:, F:2 * F] = np.asarray(b2, np.float32)[None, :]
    ballw[:, 2 * F:3 * F] = np.asarray(b3, np.float32)[None, :]
    ballw = bf(ballw)
    iota256 = bf(np.tile(np.arange(256, dtype=np.float32)[None, :], (128, 1)))
    iota64 = bf(np.tile(np.arange(F, dtype=np.float32)[None, :], (128, 1)))
    ident = bf(np.eye(128, dtype=np.float32))
    onescol = bf(np.ones((128, 1), dtype=np.float32))
    xT_bf = bf(prep["xT"])
    dinvw = bf(np.repeat(prep["dinv_lane"], F, axis=2))  # [C,128,W*F]

    in_maps = []
    for c in range(C):
        in_maps.append({
            "xT": xT_bf[c],
            "W1": W1b, "W2": W2b, "W3": W3b, "ballw": ballw,
            "iota256": iota256, "iota64": iota64, "ident": ident,
            "onescol": onescol,
            "dstl": prep["dstl_blk"][c], "wsl": prep["wsl_blk"][c],
            "idx16": prep["idx_full"][c],
            "dinv": prep["dinv_lane"][c], "dinvwide": dinvw[c],
            "batchloc": prep["bl_lane"][c],
        })

    res = run_bass_kernel_spmd(nc, in_maps, core_ids=list(range(C)))

    sums = np.zeros((N_GRAPHS, F), dtype=np.float64)
    cnts = np.zeros(N_GRAPHS, dtype=np.float64)
    for c in range(C):
        out = res.results[c]["pool_out"]
        g0 = int(prep["gmin"][c])
        for r in range(63):
            g = g0 + r
            if g < N_GRAPHS:
                sums[g] += out[r, :F]
                cnts[g] += out[r, F]
    pooled = (sums / 3.0) / np.maximum(cnts, 1.0)[:, None]
    logits = pooled @ np.asarray(Wl, np.float64) + np.asarray(bl, np.float64)
    z = logits - logits.max(axis=1, keepdims=True)
    ez = np.exp(z)
    return (ez / ez.sum(axis=1, keepdims=True)).astype(np.float32)


# revision 45
# speedup vs baseline: 1.5264x; 1.0500x over previous
"""GCN (3-layer + mean-pool + linear + softmax) on 8 Trainium2 NeuronCores.

Push-mode graph parallelism: each core owns a contiguous 12,500-node range
(padded 12544 = 128 lanes x 98 windows). Per layer, each core:
  phase A:  y~ = dinv * (x @ W) on PE, written as a bf16 table with duplicated
            rows [y~|y~] (256B rows) to DRAM,
  gathers   its OWN out-edge sources from the local table (int16 row ids,
            single chunk, large batched dma_gather ops),
  scatters  messages into per-global-window PSUM accumulators via fused
            one-hot matmuls: lhsT = (iota==dstlane)*|w| built in one
            tensor_scalar(is_equal,mult) op (DVE 4x mode / gpsimd),
  writes    bf16 partial sums [100352, 64] (partition-major rows -> large
            contiguous DMA descriptors),
  ReduceScatter (out 12544x64 bf16 ~= 55us) delivers summed aggregates for its
            own nodes; wide fused epilogue: x' = relu(dinv*(rs + y~self) + b).
Static SPMD schedule: superblocks of 7 windows with ~15 blocks each; blocks
serve a primary window and optionally the next (straddle), absorbing per-core
count variation with <=7% slot padding. Pooling via one-hot(graph) matmuls;
host applies the final 64x10 linear + softmax.
"""
import os
import sys
import numpy as np

sys.path.insert(0, os.path.dirname(os.path.abspath(__file__)))

N_NODES = 100000
N_GRAPHS = 256
IN_DIM = 128
F = 64
C = 8
NODES_C = 12500
PADN = 12544
W = 98
SBW = 7
NSB = W // SBW
GW = C * W
GSB = C * NSB
OPN = 8192
NSB_A = 10               # superblocks (of 14 per core) in the early "A" part
W_A = NSB_A * SBW        # 70 windows in A, 28 in B

_prog_cache = {}


def _win_sched(B):
    return np.minimum((np.arange(B) * SBW) // B, SBW - 1)


# --------------------------------------------------------------------------
def _split_waits(nc, cap=1):
    """Walrus rejects >1 sem wait per instruction; hoist extras onto injected
    same-engine InstEventSemaphore waits."""
    import concourse.mybir as mybir
    uid = [0]
    n_fixed = 0
    for fn in nc.m.functions:
        for bb in fn.blocks:
            insts = bb.instructions
            new_list = []
            for inst in insts:
                si = inst.sync_info
                waits = list(si.on_wait) if si and si.on_wait else []
                if len(waits) > cap:
                    extra, keep = waits[:-cap], waits[-cap:]
                    for wv in extra:
                        uid[0] += 1
                        nop = mybir.InstEventSemaphore(name=f"waitfix_{uid[0]}")
                        nop.engine = inst.engine
                        nop.sync_info = mybir.SyncInfo(on_wait=[wv], on_update=[])
                        new_list.append(nop)
                    si.on_wait = keep
                    n_fixed += 1
                new_list.append(inst)
            if len(new_list) != len(insts):
                try:
                    bb.instructions = new_list
                except Exception:
                    insts.clear()
                    insts.extend(new_list)
    return n_fixed


# --------------------------------------------------------------------------
def _host_prep(x, edge_weight, edge_index, batch):
    src = np.asarray(edge_index[0], dtype=np.int64)
    dst = np.asarray(edge_index[1], dtype=np.int64)
    w_abs = np.abs(np.asarray(edge_weight, dtype=np.float32))
    batch = np.asarray(batch, dtype=np.int64)
    x = np.asarray(x, dtype=np.float32)

    deg = np.bincount(dst, weights=w_abs.astype(np.float64), minlength=N_NODES) + 1.0
    dinv_full = (1.0 / np.sqrt(deg)).astype(np.float32)

    ks = src // NODES_C
    so = src - ks * NODES_C
    srow = ((so % 128) * W + so // 128).astype(np.int16)
    kd = dst // NODES_C
    do = dst - kd * NODES_C
    lane = (do % 128).astype(np.float32)
    g = kd * W + do // 128

    # superblock processing order: A-half (u<49) sbs of all cores first
    order_sb = sorted(range(GSB), key=lambda s: ((s % NSB) >= NSB_A, s))
    rank_of = np.empty(GSB, dtype=np.int64)
    for r, s in enumerate(order_sb):
        rank_of[s] = r

    gsb = g // SBW
    key = (ks * GSB + rank_of[gsb]) * SBW + (g % SBW)
    order = np.argsort(key, kind="stable")
    ks_s, g_s = ks[order], g[order]
    srow_s, lane_s, w_s = srow[order], lane[order], w_abs[order]

    cnt_cw = np.zeros((C, GW), dtype=np.int64)
    np.add.at(cnt_cw, (ks_s, g_s), 1)
    cnt_csb = cnt_cw.reshape(C, GSB, SBW).sum(axis=2)

    B_sb = np.maximum(np.ceil(cnt_csb.max(axis=0) / 128).astype(np.int64), SBW)

    def feasible(sb, B):
        w1 = _win_sched(B)
        firsts = [np.where((w1 == v) | (w1 == v - 1))[0][0] for v in range(SBW)]
        lasts = [np.where(w1 == v)[0][-1] for v in range(SBW)]
        for c in range(C):
            pos = 0
            for v in range(SBW):
                pos = max(pos, int(firsts[v]) * 128)
                pos += cnt_cw[c, sb * SBW + v]
                if pos > (int(lasts[v]) + 1) * 128:
                    return False
        return True

    for sb in range(GSB):
        while not feasible(sb, int(B_sb[sb])):
            B_sb[sb] += 1

    B_rank = np.array([B_sb[s] for s in order_sb])
    blk_of_rank = np.concatenate([[0], np.cumsum(B_rank)])
    NBLK = int(blk_of_rank[-1])
    TOT = NBLK * 128
    nA_blk = int(blk_of_rank[C * NSB_A])  # blocks in the A part

    flat_cnt = np.zeros(C * GW, dtype=np.int64)
    np.add.at(flat_cnt, ks_s * GW + g_s, 1)
    flat_start = np.concatenate([[0], np.cumsum(flat_cnt)])

    idx_slot = np.zeros((C, TOT), dtype=np.int16)
    w_slot = np.zeros((C, TOT), dtype=np.float32)
    win_slot = np.full((C, TOT), -1, dtype=np.int64)
    lane_slot = np.full((C, TOT), -1.0, dtype=np.float32)

    for r in range(GSB):
        sb = order_sb[r]
        B = int(B_sb[sb])
        w1 = _win_sched(B)
        base = blk_of_rank[r] * 128
        firsts = [int(np.where((w1 == v) | (w1 == v - 1))[0][0]) for v in range(SBW)]
        for c in range(C):
            pos = 0
            for v in range(SBW):
                gidx = sb * SBW + v
                n = cnt_cw[c, gidx]
                pos = max(pos, firsts[v] * 128)
                if n:
                    e0 = flat_start[c * GW + gidx]
                    sl = slice(base + pos, base + pos + n)
                    idx_slot[c, sl] = srow_s[e0:e0 + n]
                    w_slot[c, sl] = w_s[e0:e0 + n]
                    win_slot[c, sl] = gidx
                    lane_slot[c, sl] = lane_s[e0:e0 + n]
                    pos += n

    prim_w1 = np.empty(NBLK, dtype=np.int64)
    for r in range(GSB):
        sb = order_sb[r]
        B = int(B_sb[sb])
        w1 = _win_sched(B)
        prim_w1[blk_of_rank[r]:blk_of_rank[r] + B] = sb * SBW + w1

    sec_needed = np.zeros(NBLK, dtype=bool)
    blk_of_slot = np.arange(TOT) // 128
    for c in range(C):
        m = win_slot[c] >= 0
        sec = win_slot[c][m] != prim_w1[blk_of_slot[m]]
        np.logical_or.at(sec_needed, blk_of_slot[m][sec], True)

    mm_block, mm_win = [], []
    for blk in range(NBLK):
        mm_block.append(blk); mm_win.append(int(prim_w1[blk]))
        if sec_needed[blk]:
            mm_block.append(blk); mm_win.append(int(prim_w1[blk]) + 1)
    mm_block = np.array(mm_block); mm_win = np.array(mm_win)
    NMM = len(mm_block)

    first_mm, last_mm = {}, {}
    for m in range(NMM):
        wn = int(mm_win[m])
        if wn not in first_mm:
            first_mm[wn] = m
        last_mm[wn] = m
    assert len(first_mm) == GW

    # per-block lane encoding vs the block's primary window:
    # lane + 128*(win - w1) for win in {w1, w1+1}, else -1
    ls = lane_slot.reshape(C, NBLK, 128)
    vs = win_slot.reshape(C, NBLK, 128)
    rel = vs - prim_w1[None, :, None]
    valid = (rel == 0) | (rel == 1)
    enc = np.where(valid, ls + 128.0 * rel, -1.0).astype(np.float32)
    dstl_blk = enc.transpose(0, 2, 1).copy()          # [C, 128, NBLK]
    wsl_blk = w_slot.reshape(C, NBLK, 128).transpose(0, 2, 1).copy()

    n_ops = (TOT + OPN - 1) // OPN
    op_sizes = [min(OPN, TOT - i * OPN) for i in range(n_ops)]
    idx_wrap = np.zeros((C, 16, TOT // 16), dtype=np.int16)
    off = 0
    for s in op_sizes:
        seg = idx_slot[:, off:off + s].reshape(C, s // 16, 16)
        idx_wrap[:, :, off // 16:(off + s) // 16] = seg.transpose(0, 2, 1)
        off += s
    idx_full = np.tile(idx_wrap, (1, 8, 1))

    o = np.arange(NODES_C)
    u_of = o // 128
    p_of = o % 128
    dinv_lane = np.ones((C, 128, W), dtype=np.float32)
    bl_lane = np.full((C, 128, W), 63.0, dtype=np.float32)
    gmin = np.zeros(C, dtype=np.int64)
    xT = np.zeros((C, IN_DIM, PADN), dtype=np.float32)
    for c in range(C):
        n0 = c * NODES_C
        dinv_lane[c, p_of, u_of] = dinv_full[n0:n0 + NODES_C]
        bseg = batch[n0:n0 + NODES_C]
        gmin[c] = bseg[0]
        assert int(bseg[-1] - bseg[0]) <= 62
        bl_lane[c, p_of, u_of] = (bseg - gmin[c]).astype(np.float32)
        xT[c, :, :NODES_C] = x[n0:n0 + NODES_C].T

    return dict(
        B_sb=B_sb, NBLK=NBLK, TOT=TOT, NMM=NMM, nA_blk=nA_blk,
        mm_block=mm_block, mm_win=mm_win, first_mm=first_mm, last_mm=last_mm,
        sec_needed=sec_needed, prim_w1=prim_w1,
        dstl_blk=dstl_blk, wsl_blk=wsl_blk, idx_full=idx_full,
        dinv_lane=dinv_lane, bl_lane=bl_lane, gmin=gmin, xT=xT,
        op_sizes=op_sizes,
        idx_slot=idx_slot, w_slot=w_slot, win_slot=win_slot,
        lane_slot=lane_slot,
    )


# --------------------------------------------------------------------------
def _build_program(prep, has_bias=True):
    import concourse.bacc as bacc
    import concourse.mybir as mybir
    import concourse.tile as tile
    from contextlib import ExitStack

    f32 = mybir.dt.float32
    bf16 = mybir.dt.bfloat16
    i16 = mybir.dt.int16
    OP = mybir.AluOpType
    AF = mybir.ActivationFunctionType

    NBLK = prep["NBLK"]
    TOT = prep["TOT"]
    NMM = prep["NMM"]
    mm_block = prep["mm_block"]
    mm_win = prep["mm_win"]
    first_mm = prep["first_mm"]
    last_mm = prep["last_mm"]
    op_sizes = prep["op_sizes"]

    nc = bacc.Bacc("TRN2", target_bir_lowering=False, debug=False, num_devices=C)

    xT_in = nc.declare_dram_parameter("xT", [IN_DIM, PADN], bf16, isOutput=False)
    W1_in = nc.declare_dram_parameter("W1", [IN_DIM, F], bf16, isOutput=False)
    W2_in = nc.declare_dram_parameter("W2", [F, F], bf16, isOutput=False)
    W3_in = nc.declare_dram_parameter("W3", [F, F], bf16, isOutput=False)
    ball_in = nc.declare_dram_parameter("ballw", [128, 3 * F], bf16, isOutput=False)
    iota256_in = nc.declare_dram_parameter("iota256", [128, 256], bf16, isOutput=False)
    iota64_in = nc.declare_dram_parameter("iota64", [128, F], bf16, isOutput=False)
    ident_in = nc.declare_dram_parameter("ident", [128, 128], bf16, isOutput=False)
    ones_in = nc.declare_dram_parameter("onescol", [128, 1], bf16, isOutput=False)
    dstl_in = nc.declare_dram_parameter("dstl", [128, NBLK], f32, isOutput=False)
    wsl_in = nc.declare_dram_parameter("wsl", [128, NBLK], f32, isOutput=False)
    idx_in = nc.declare_dram_parameter("idx16", [128, TOT // 16], i16, isOutput=False)
    dinv_in = nc.declare_dram_parameter("dinv", [128, W], f32, isOutput=False)
    dinvw_in = nc.declare_dram_parameter("dinvwide", [128, W * F], bf16, isOutput=False)
    bl_in = nc.declare_dram_parameter("batchloc", [128, W], f32, isOutput=False)
    pool_out = nc.declare_dram_parameter("pool_out", [F, F + 1], f32, isOutput=True)

    stk = ExitStack()
    tbl_sems = [stk.enter_context(nc.semaphore(f"tbl_{i}")) for i in range(3)]
    wrA_sems = [stk.enter_context(nc.semaphore(f"wrA_{i}")) for i in range(3)]
    wrB_sems = [stk.enter_context(nc.semaphore(f"wrB_{i}")) for i in range(3)]
    ccA_sems = [stk.enter_context(nc.semaphore(f"ccA_{i}")) for i in range(3)]
    ccB_sems = [stk.enter_context(nc.semaphore(f"ccB_{i}")) for i in range(3)]

    HW = W_A                   # 70 windows in A, 28 in B
    HN = W_A * 128
    GRP0_H = ([0, 14, 28, 42, 56], [0, 14])
    GRPS_H = ([14] * 5, [14] * 2)
    N_WR_H = (len(GRP0_H[0]) * C, len(GRP0_H[1]) * C)

    with tile.TileContext(nc, num_cores=C) as tc:
        tc.race_detector_enabled = False
        with (
            tc.tile_pool(name="persist", bufs=1) as pp,
            tc.tile_pool(name="idxp", bufs=3) as idxp,
            tc.tile_pool(name="msgp", bufs=3) as mp,
            tc.tile_pool(name="wstp", bufs=16) as wp,
            tc.tile_pool(name="stgp", bufs=3) as sgp,
            tc.tile_pool(name="epi", bufs=1) as ep,
            tc.tile_pool(name="ps", bufs=7, space="PSUM") as ps,
            tc.tile_pool(name="psPool", bufs=1, space="PSUM") as ps1,
            tc.tile_pool(name="dram", bufs=1, space="DRAM") as dr,
        ):
            def load(name, shape, dt, src):
                t = pp.tile(shape, dt, name=name)
                nc.sync.dma_start(out=t[:], in_=src[:])
                return t

            xT_a = load("xT_a", [IN_DIM, PADN], bf16, xT_in)
            w1 = load("w1", [IN_DIM, F], bf16, W1_in)
            w2 = load("w2", [F, F], bf16, W2_in)
            w3 = load("w3", [F, F], bf16, W3_in)
            ballw = load("ballw", [128, 3 * F], bf16, ball_in)
            iota256 = load("iota256", [128, 256], bf16, iota256_in)
            iota64 = load("iota64", [128, F], bf16, iota64_in)
            ident = load("ident", [128, 128], bf16, ident_in)
            onescol = load("onescol", [128, 1], bf16, ones_in)
            dstl = load("dstl", [128, NBLK], f32, dstl_in)
            wsl = load("wsl", [128, NBLK], f32, wsl_in)
            dinv = load("dinv", [128, W], f32, dinv_in)
            dinvw = load("dinvwide", [128, W * F], bf16, dinvw_in)
            batchloc = load("batchloc", [128, W], f32, bl_in)

            ytb = pp.tile([128, W * 128], bf16, name="ytb")
            xTn = pp.tile([F, PADN], bf16, name="xTn")
            acc = pp.tile([128, W * F], bf16, name="acc")
            xpr = pp.tile([128, W * F], bf16, name="xpr")
            rs_sbA = pp.tile([128, HW * F], bf16, name="rs_sbA")
            rs_sbB = pp.tile([128, (W - HW) * F], bf16, name="rs_sbB")

            psum_pool = ps1.tile([F, F + 1], f32, name="psum_pool")
            psum_sums = psum_pool[:, 0:F]
            psum_cnt = psum_pool[:, F:F + 1]
            pool_mm = [0]  # matmul counter for start flags

            tables = [dr.tile([PADN, 128], bf16, name=f"table_{L}")
                      for L in range(3)]
            partsA = [dr.tile([C * HN, F], bf16, name=f"partA_{L}")
                      for L in range(3)]
            partsB = [dr.tile([C * (PADN - HN), F], bf16, name=f"partB_{L}")
                      for L in range(3)]
            rsoutA = [dr.tile([HN, F], bf16, name=f"rsoutA_{L}")
                      for L in range(3)]
            rsoutB = [dr.tile([PADN - HN, F], bf16, name=f"rsoutB_{L}")
                      for L in range(3)]

            def phase_a(L, u0, u1):
                """y~ = dinv*(x@W) for windows [u0,u1); dup bf16 into ytb;
                then write the table rows for that half."""
                wmat = (w1, w2, w3)[L]
                for u in range(u0, u1):
                    psum_y = ps.tile([128, F], f32, name="psum_y", tag="psum_y",
                                     bufs=2)
                    lhsT = (xT_a if L == 0 else xTn)[:, u * 128:(u + 1) * 128]
                    nc.tensor.matmul(psum_y[:], lhsT, wmat[:], start=True,
                                     stop=True)
                    nc.vector.tensor_scalar(
                        ytb[:, u * 128:u * 128 + F], psum_y[:],
                        dinv[:, u:u + 1], None, OP.mult)
                    nc.scalar.activation(
                        ytb[:, u * 128 + F:(u + 1) * 128], psum_y[:],
                        AF.Copy, scale=dinv[:, u:u + 1])
                tbl_ap = tables[L][:].rearrange("(p u) e -> p u e", p=128)
                nc.sync.dma_start(
                    out=tbl_ap[:, u0:u1, :],
                    in_=ytb[:, u0 * 128:u1 * 128].rearrange(
                        "p (u e) -> p u e", e=128),
                ).then_inc(tbl_sems[L], 16)

            def epilogue(L, half):
                """x' = relu(dinv*(rs + y~self) + b) for one half; acc/pool."""
                u0 = 0 if half == 0 else HW
                u1 = HW if half == 0 else W
                nw = u1 - u0
                rs_sb = rs_sbA if half == 0 else rs_sbB
                rso = (rsoutA if half == 0 else rsoutB)[L]
                cc = (ccA_sems if half == 0 else ccB_sems)[L]
                nc.sync.wait_ge(cc, 1)
                nc.sync.dma_start(
                    out=rs_sb[:],
                    in_=rso[:].rearrange("(p u) f -> p (u f)", p=128))
                xs = xpr[:, u0 * F:u1 * F]
                ytb_self = ytb[:].rearrange(
                    "p (u e) -> p u e", e=128)[:, u0:u1, 0:F]
                nc.vector.tensor_tensor(xs, rs_sb[:], ytb_self, OP.add)
                nc.vector.tensor_tensor(xs, xs, dinvw[:, u0 * F:u1 * F],
                                        OP.mult)
                if has_bias:
                    for u in range(u0, u1):
                        nc.vector.tensor_tensor(
                            xpr[:, u * F:(u + 1) * F],
                            xpr[:, u * F:(u + 1) * F],
                            ballw[:, L * F:(L + 1) * F], OP.add)
                nc.vector.tensor_scalar(xs, xs, 0.0, None, OP.max)
                if L == 0:
                    nc.vector.tensor_copy(acc[:, u0 * F:u1 * F], xs)
                elif L == 1:
                    nc.vector.tensor_tensor(acc[:, u0 * F:u1 * F],
                                            acc[:, u0 * F:u1 * F], xs, OP.add)
                if L < 2:
                    for u in range(u0, u1):
                        ptr = ps.tile([F, 128], bf16, name="ptr", tag="ptr",
                                      bufs=2)
                        nc.tensor.transpose(ptr[:], xpr[:, u * F:(u + 1) * F],
                                            ident[:])
                        eng = nc.gpsimd if u % 2 == 0 else nc.vector
                        eng.tensor_copy(xTn[:, u * 128:(u + 1) * 128], ptr[:])
                    phase_a(L + 1, u0, u1)
                else:
                    pool_pass(xpr, u0, u1, last=(half == 1))

            def pool_pass(src_tile, u0, u1, last):
                for u in range(u0, u1):
                    sg = wp.tile([128, 256], bf16, name="sg", tag="wst")
                    nc.vector.tensor_scalar(
                        sg[:, :F], iota64[:], batchloc[:, u:u + 1], None,
                        OP.is_equal)
                    first = pool_mm[0] == 0
                    stop = last and (u == u1 - 1)
                    nc.tensor.matmul(
                        psum_sums, sg[:, :F], src_tile[:, u * F:(u + 1) * F],
                        start=first, stop=stop, skip_group_check=True)
                    nc.tensor.matmul(
                        psum_cnt, sg[:, :F], onescol[:],
                        start=first, stop=stop, skip_group_check=True)
                    pool_mm[0] += 1

            # ---------------- initial phase A (layer 0) ---------------------
            phase_a(0, 0, HW)
            phase_a(0, HW, W)

            for L in range(3):
                partial_h = (partsA[L], partsB[L])
                rsout_h = (rsoutA[L], rsoutB[L])
                wr_h = (wrA_sems[L], wrB_sems[L])
                cc_h = (ccA_sems[L], ccB_sems[L])

                nc.gpsimd.wait_ge(tbl_sems[L], 32)

                # pooling pass 1 over acc = x1+x2 runs during layer-2 blocks
                if L == 2:
                    pool_pass(acc, 0, W, last=False)

                n_ops = len(op_sizes)
                mts = [None] * n_ops
                op_base_blk = [0] * n_ops
                off = 0
                for k, s in enumerate(op_sizes):
                    op_base_blk[k] = off // 128
                    off += s

                size_regs = {}
                for s in set(op_sizes):
                    size_regs[s] = nc.gpsimd.to_reg(s)

                def issue_gather(k):
                    s = op_sizes[k]
                    off16 = sum(op_sizes[:k]) // 16
                    it = idxp.tile([128, OPN // 16], i16, name="idxt",
                                   tag="idxt")
                    nc.gpsimd.dma_start(out=it[:, :s // 16],
                                        in_=idx_in[:, off16:off16 + s // 16])
                    mt = mp.tile([128, OPN // 128, 128], bf16, name="msg",
                                 tag="msg")
                    nc.gpsimd.dma_gather(
                        out_ap=mt[:, :s // 128, :],
                        in_ap=tables[L][:],
                        idxs_ap=it[:, :s // 16],
                        num_idxs=s,
                        num_idxs_reg=size_regs[s],
                        elem_size=128,
                    )
                    mts[k] = mt

                issue_gather(0)
                if n_ops > 1:
                    issue_gather(1)

                psums = {}
                stg = {}

                def window_close(wn):
                    u = wn % W
                    kk = wn // W
                    half = 0 if u < HW else 1
                    uu = u - (0 if half == 0 else HW)
                    grp = min(uu // 14, len(GRP0_H[half]) - 1)
                    gkey = (kk, half, grp)
                    sz = GRPS_H[half][grp]
                    if gkey not in stg:
                        stg[gkey] = sgp.tile([128, 14 * F], bf16, name="stg",
                                             tag="stg")
                    voff = uu - GRP0_H[half][grp]
                    nc.scalar.activation(
                        stg[gkey][:, voff * F:(voff + 1) * F],
                        psums.pop(wn)[:], AF.Copy)
                    if voff == sz - 1:
                        part = partial_h[half]
                        nh = HW if half == 0 else W - HW
                        g0 = GRP0_H[half][grp]
                        dst_ap = part[:].rearrange(
                            "(k p u) f -> k p (u f)", k=C, p=128
                        )[kk][:, g0 * F:(g0 + sz) * F]
                        nc.sync.dma_start(
                            out=dst_ap, in_=stg.pop(gkey)[:, :sz * F]
                        ).then_inc(wr_h[half], 16)

                nA_blk = prep["nA_blk"]
                trigger_blk = nA_blk + int(0.55 * (NBLK - nA_blk))
                closedA = [0]

                m = 0
                cur_op = 0
                for blk in range(NBLK):
                    while cur_op + 1 < n_ops and blk >= op_base_blk[cur_op + 1]:
                        cur_op += 1
                    for ahead in (1, 2):
                        if cur_op + ahead < n_ops and mts[cur_op + ahead] is None:
                            issue_gather(cur_op + ahead)
                    mt = mts[cur_op]
                    jloc = blk - op_base_blk[cur_op]

                    sec = bool(prep["sec_needed"][blk])
                    width = 256 if sec else 128
                    wst = wp.tile([128, 256], bf16, name="wst", tag="wst")
                    nc.vector.tensor_scalar(
                        wst[:, :width], iota256[:, :width],
                        dstl[:, blk:blk + 1], wsl[:, blk:blk + 1],
                        OP.is_equal, OP.mult)

                    for half_mm in range(2 if sec else 1):
                        wn = int(mm_win[m])
                        if wn not in psums:
                            psums[wn] = ps.tile([128, F], f32, name="psum_w",
                                                tag="psum_w", bufs=3)
                        nc.tensor.matmul(
                            psums[wn][:],
                            wst[:, half_mm * 128:(half_mm + 1) * 128],
                            mt[:, jloc, 0:F],
                            start=(m == first_mm[wn]), stop=(m == last_mm[wn]),
                            skip_group_check=True)
                        m += 1
                        if (m - 1) == last_mm[wn]:
                            window_close(wn)
                            if (wn % W) < HW:
                                closedA[0] += 1
                                if closedA[0] == HW * C:
                                    # all A windows staged: kick RS_A
                                    nc.gpsimd.wait_ge(wr_h[0], 16 * N_WR_H[0])
                                    nc.gpsimd.collective_compute(
                                        "ReduceScatter", OP.add,
                                        replica_groups=[list(range(C))],
                                        ins=[partial_h[0][:]],
                                        outs=[rsout_h[0][:]],
                                    ).then_inc(cc_h[0], 1)

                    if blk == trigger_blk:
                        epilogue(L, 0)

                assert m == NMM

                # RS_B + exposed B boundary
                nc.gpsimd.wait_ge(wr_h[1], 16 * N_WR_H[1])
                nc.gpsimd.collective_compute(
                    "ReduceScatter", OP.add,
                    replica_groups=[list(range(C))],
                    ins=[partial_h[1][:]],
                    outs=[rsout_h[1][:]],
                ).then_inc(cc_h[1], 1)
                epilogue(L, 1)

            # ---------------- pooling output ---------------------------------
            outt = ep.tile([F, F + 1], f32, name="outt")
            nc.vector.tensor_copy(outt[:, :F], psum_sums)
            nc.vector.tensor_copy(outt[:, F:F + 1], psum_cnt)
            nc.sync.dma_start(out=pool_out[:], in_=outt[:])

    stk.close()
    nc.compile()
    _split_waits(nc)
    return nc


# --------------------------------------------------------------------------
def kernel(x, edge_weight, W1, b1, W2, b2, W3, b3, Wl, bl, edge_index, batch):
    from concourse.bass_utils import run_bass_kernel_spmd
    import jax.numpy as jnp

    prep = _host_prep(x, edge_weight, edge_index, batch)

    has_bias = any(np.any(np.asarray(b) != 0) for b in (b1, b2, b3))
    cache_key = (prep["NBLK"], prep["NMM"], tuple(prep["op_sizes"][:3]), has_bias)
    if cache_key not in _prog_cache:
        _prog_cache[cache_key] = _build_program(prep, has_bias=has_bias)
    nc = _prog_cache[cache_key]

    bf = lambda a: np.asarray(jnp.asarray(np.asarray(a, np.float32), jnp.bfloat16))
    W1b, W2b, W3b = bf(W1), bf(W2), bf(W3)
    ballw = np.zeros((128, 3 * F), dtype=np.float32)
    ballw[:, 0:F] = np.asarray(b1, np.float32)[None, :]
    ballw[:, F:2 * F] = np.asarray(b2, np.float32)[None, :]
    ballw[:, 2 * F:3 * F] = np.asarray(b3, np.float32)[None, :]
    ballw = bf(ballw)
    iota256 = bf(np.tile(np.arange(256, dtype=np.float32)[None, :], (128, 1)))
    iota64 = bf(np.tile(np.arange(F, dtype=np.float32)[None, :], (128, 1)))
    ident = bf(np.eye(128, dtype=np.float32))
    onescol = bf(np.ones((128, 1), dtype=np.float32))
    xT_bf = bf(prep["xT"])
    dinvw = bf(np.repeat(prep["dinv_lane"], F, axis=2))  # [C,128,W*F]

    in_maps = []
    for c in range(C):
        in_maps.append({
            "xT": xT_bf[c],
            "W1": W1b, "W2": W2b, "W3": W3b, "ballw": ballw,
            "iota256": iota256, "iota64": iota64, "ident": ident,
            "onescol": onescol,
            "dstl": prep["dstl_blk"][c], "wsl": prep["wsl_blk"][c],
            "idx16": prep["idx_full"][c],
            "dinv": prep["dinv_lane"][c], "dinvwide": dinvw[c],
            "batchloc": prep["bl_lane"][c],
        })

    res = run_bass_kernel_spmd(nc, in_maps, core_ids=list(range(C)))

    sums = np.zeros((N_GRAPHS, F), dtype=np.float64)
    cnts = np.zeros(N_GRAPHS, dtype=np.float64)
    for c in range(C):
        out = res.results[c]["pool_out"]
        g0 = int(prep["gmin"][c])
        for r in range(63):
            g = g0 + r
            if g < N_GRAPHS:
                sums[g] += out[r, :F]
                cnts[g] += out[r, F]
    pooled = (sums / 3.0) / np.maximum(cnts, 1.0)[:, None]
    logits = pooled @ np.asarray(Wl, np.float64) + np.asarray(bl, np.float64)
    z = logits - logits.max(axis=1, keepdims=True)
    ez = np.exp(z)
    return (ez / ez.sum(axis=1, keepdims=True)).astype(np.float32)


# revision 47
# speedup vs baseline: 1.5628x; 1.0238x over previous
"""GCN (3-layer + mean-pool + linear + softmax) on 8 Trainium2 NeuronCores.

Push-mode graph parallelism: each core owns a contiguous 12,500-node range
(padded 12544 = 128 lanes x 98 windows). Per layer, each core:
  phase A:  y~ = dinv * (x @ W) on PE, written as a bf16 table with duplicated
            rows [y~|y~] (256B rows) to DRAM,
  gathers   its OWN out-edge sources from the local table (int16 row ids,
            single chunk, large batched dma_gather ops),
  scatters  messages into per-global-window PSUM accumulators via fused
            one-hot matmuls: lhsT = (iota==dstlane)*|w| built in one
            tensor_scalar(is_equal,mult) op (DVE 4x mode / gpsimd),
  writes    bf16 partial sums [100352, 64] (partition-major rows -> large
            contiguous DMA descriptors),
  ReduceScatter (out 12544x64 bf16 ~= 55us) delivers summed aggregates for its
            own nodes; wide fused epilogue: x' = relu(dinv*(rs + y~self) + b).
Static SPMD schedule: superblocks of 7 windows with ~15 blocks each; blocks
serve a primary window and optionally the next (straddle), absorbing per-core
count variation with <=7% slot padding. Pooling via one-hot(graph) matmuls;
host applies the final 64x10 linear + softmax.
"""
import os
import sys
import numpy as np

sys.path.insert(0, os.path.dirname(os.path.abspath(__file__)))

N_NODES = 100000
N_GRAPHS = 256
IN_DIM = 128
F = 64
C = 8
NODES_C = 12500
PADN = 12544
W = 98
SBW = 7
NSB = W // SBW
GW = C * W
GSB = C * NSB
OPN = 8192
NSB_A = 10               # superblocks (of 14 per core) in the early "A" part
W_A = NSB_A * SBW        # 70 windows in A, 28 in B

_prog_cache = {}


def _win_sched(B):
    return np.minimum((np.arange(B) * SBW) // B, SBW - 1)


# --------------------------------------------------------------------------
def _split_waits(nc, cap=1):
    """Walrus rejects >1 sem wait per instruction; hoist extras onto injected
    same-engine InstEventSemaphore waits."""
    import concourse.mybir as mybir
    uid = [0]
    n_fixed = 0
    for fn in nc.m.functions:
        for bb in fn.blocks:
            insts = bb.instructions
            new_list = []
            for inst in insts:
                si = inst.sync_info
                waits = list(si.on_wait) if si and si.on_wait else []
                if len(waits) > cap:
                    extra, keep = waits[:-cap], waits[-cap:]
                    for wv in extra:
                        uid[0] += 1
                        nop = mybir.InstEventSemaphore(name=f"waitfix_{uid[0]}")
                        nop.engine = inst.engine
                        nop.sync_info = mybir.SyncInfo(on_wait=[wv], on_update=[])
                        new_list.append(nop)
                    si.on_wait = keep
                    n_fixed += 1
                new_list.append(inst)
            if len(new_list) != len(insts):
                try:
                    bb.instructions = new_list
                except Exception:
                    insts.clear()
                    insts.extend(new_list)
    return n_fixed


# --------------------------------------------------------------------------
def _host_prep(x, edge_weight, edge_index, batch):
    src = np.asarray(edge_index[0], dtype=np.int64)
    dst = np.asarray(edge_index[1], dtype=np.int64)
    w_abs = np.abs(np.asarray(edge_weight, dtype=np.float32))
    batch = np.asarray(batch, dtype=np.int64)
    x = np.asarray(x, dtype=np.float32)

    deg = np.bincount(dst, weights=w_abs.astype(np.float64), minlength=N_NODES) + 1.0
    dinv_full = (1.0 / np.sqrt(deg)).astype(np.float32)

    ks = src // NODES_C
    so = src - ks * NODES_C
    srow = ((so % 128) * W + so // 128).astype(np.int16)
    kd = dst // NODES_C
    do = dst - kd * NODES_C
    lane = (do % 128).astype(np.float32)
    g = kd * W + do // 128

    # superblock processing order: A-half (u<49) sbs of all cores first
    order_sb = sorted(range(GSB), key=lambda s: ((s % NSB) >= NSB_A, s))
    rank_of = np.empty(GSB, dtype=np.int64)
    for r, s in enumerate(order_sb):
        rank_of[s] = r

    gsb = g // SBW
    key = (ks * GSB + rank_of[gsb]) * SBW + (g % SBW)
    order = np.argsort(key, kind="stable")
    ks_s, g_s = ks[order], g[order]
    srow_s, lane_s, w_s = srow[order], lane[order], w_abs[order]

    cnt_cw = np.zeros((C, GW), dtype=np.int64)
    np.add.at(cnt_cw, (ks_s, g_s), 1)
    cnt_csb = cnt_cw.reshape(C, GSB, SBW).sum(axis=2)

    B_sb = np.maximum(np.ceil(cnt_csb.max(axis=0) / 128).astype(np.int64), SBW)

    def feasible(sb, B):
        w1 = _win_sched(B)
        firsts = [np.where((w1 == v) | (w1 == v - 1))[0][0] for v in range(SBW)]
        lasts = [np.where(w1 == v)[0][-1] for v in range(SBW)]
        for c in range(C):
            pos = 0
            for v in range(SBW):
                pos = max(pos, int(firsts[v]) * 128)
                pos += cnt_cw[c, sb * SBW + v]
                if pos > (int(lasts[v]) + 1) * 128:
                    return False
        return True

    for sb in range(GSB):
        while not feasible(sb, int(B_sb[sb])):
            B_sb[sb] += 1

    B_rank = np.array([B_sb[s] for s in order_sb])
    blk_of_rank = np.concatenate([[0], np.cumsum(B_rank)])
    NBLK = int(blk_of_rank[-1])
    TOT = NBLK * 128
    nA_blk = int(blk_of_rank[C * NSB_A])  # blocks in the A part

    flat_cnt = np.zeros(C * GW, dtype=np.int64)
    np.add.at(flat_cnt, ks_s * GW + g_s, 1)
    flat_start = np.concatenate([[0], np.cumsum(flat_cnt)])

    idx_slot = np.zeros((C, TOT), dtype=np.int16)
    w_slot = np.zeros((C, TOT), dtype=np.float32)
    win_slot = np.full((C, TOT), -1, dtype=np.int64)
    lane_slot = np.full((C, TOT), -1.0, dtype=np.float32)

    for r in range(GSB):
        sb = order_sb[r]
        B = int(B_sb[sb])
        w1 = _win_sched(B)
        base = blk_of_rank[r] * 128
        firsts = [int(np.where((w1 == v) | (w1 == v - 1))[0][0]) for v in range(SBW)]
        for c in range(C):
            pos = 0
            for v in range(SBW):
                gidx = sb * SBW + v
                n = cnt_cw[c, gidx]
                pos = max(pos, firsts[v] * 128)
                if n:
                    e0 = flat_start[c * GW + gidx]
                    sl = slice(base + pos, base + pos + n)
                    idx_slot[c, sl] = srow_s[e0:e0 + n]
                    w_slot[c, sl] = w_s[e0:e0 + n]
                    win_slot[c, sl] = gidx
                    lane_slot[c, sl] = lane_s[e0:e0 + n]
                    pos += n

    prim_w1 = np.empty(NBLK, dtype=np.int64)
    for r in range(GSB):
        sb = order_sb[r]
        B = int(B_sb[sb])
        w1 = _win_sched(B)
        prim_w1[blk_of_rank[r]:blk_of_rank[r] + B] = sb * SBW + w1

    sec_needed = np.zeros(NBLK, dtype=bool)
    blk_of_slot = np.arange(TOT) // 128
    for c in range(C):
        m = win_slot[c] >= 0
        sec = win_slot[c][m] != prim_w1[blk_of_slot[m]]
        np.logical_or.at(sec_needed, blk_of_slot[m][sec], True)

    mm_block, mm_win = [], []
    for blk in range(NBLK):
        mm_block.append(blk); mm_win.append(int(prim_w1[blk]))
        if sec_needed[blk]:
            mm_block.append(blk); mm_win.append(int(prim_w1[blk]) + 1)
    mm_block = np.array(mm_block); mm_win = np.array(mm_win)
    NMM = len(mm_block)

    first_mm, last_mm = {}, {}
    for m in range(NMM):
        wn = int(mm_win[m])
        if wn not in first_mm:
            first_mm[wn] = m
        last_mm[wn] = m
    assert len(first_mm) == GW

    # per-block lane encoding vs the block's primary window:
    # lane + 128*(win - w1) for win in {w1, w1+1}, else -1
    ls = lane_slot.reshape(C, NBLK, 128)
    vs = win_slot.reshape(C, NBLK, 128)
    rel = vs - prim_w1[None, :, None]
    valid = (rel == 0) | (rel == 1)
    enc = np.where(valid, ls + 128.0 * rel, -1.0).astype(np.float32)
    dstl_blk = enc.transpose(0, 2, 1).copy()          # [C, 128, NBLK]
    wsl_blk = w_slot.reshape(C, NBLK, 128).transpose(0, 2, 1).copy()

    n_ops = (TOT + OPN - 1) // OPN
    op_sizes = [min(OPN, TOT - i * OPN) for i in range(n_ops)]
    idx_wrap = np.zeros((C, 16, TOT // 16), dtype=np.int16)
    off = 0
    for s in op_sizes:
        seg = idx_slot[:, off:off + s].reshape(C, s // 16, 16)
        idx_wrap[:, :, off // 16:(off + s) // 16] = seg.transpose(0, 2, 1)
        off += s
    idx_full = np.tile(idx_wrap, (1, 8, 1))

    o = np.arange(NODES_C)
    u_of = o // 128
    p_of = o % 128
    dinv_lane = np.ones((C, 128, W), dtype=np.float32)
    bl_lane = np.full((C, 128, W), 63.0, dtype=np.float32)
    gmin = np.zeros(C, dtype=np.int64)
    xT = np.zeros((C, IN_DIM, PADN), dtype=np.float32)
    for c in range(C):
        n0 = c * NODES_C
        dinv_lane[c, p_of, u_of] = dinv_full[n0:n0 + NODES_C]
        bseg = batch[n0:n0 + NODES_C]
        gmin[c] = bseg[0]
        assert int(bseg[-1] - bseg[0]) <= 62
        bl_lane[c, p_of, u_of] = (bseg - gmin[c]).astype(np.float32)
        xT[c, :, :NODES_C] = x[n0:n0 + NODES_C].T

    return dict(
        B_sb=B_sb, NBLK=NBLK, TOT=TOT, NMM=NMM, nA_blk=nA_blk,
        mm_block=mm_block, mm_win=mm_win, first_mm=first_mm, last_mm=last_mm,
        sec_needed=sec_needed, prim_w1=prim_w1,
        dstl_blk=dstl_blk, wsl_blk=wsl_blk, idx_full=idx_full,
        dinv_lane=dinv_lane, bl_lane=bl_lane, gmin=gmin, xT=xT,
        op_sizes=op_sizes,
        idx_slot=idx_slot, w_slot=w_slot, win_slot=win_slot,
        lane_slot=lane_slot,
    )


# --------------------------------------------------------------------------
def _build_program(prep, has_bias=True):
    import concourse.bacc as bacc
    import concourse.mybir as mybir
    import concourse.tile as tile
    from contextlib import ExitStack

    f32 = mybir.dt.float32
    bf16 = mybir.dt.bfloat16
    i16 = mybir.dt.int16
    OP = mybir.AluOpType
    AF = mybir.ActivationFunctionType

    NBLK = prep["NBLK"]
    TOT = prep["TOT"]
    NMM = prep["NMM"]
    mm_block = prep["mm_block"]
    mm_win = prep["mm_win"]
    first_mm = prep["first_mm"]
    last_mm = prep["last_mm"]
    op_sizes = prep["op_sizes"]

    nc = bacc.Bacc("TRN2", target_bir_lowering=False, debug=False, num_devices=C)

    xT_in = nc.declare_dram_parameter("xT", [IN_DIM, PADN], bf16, isOutput=False)
    W1_in = nc.declare_dram_parameter("W1", [IN_DIM, F], bf16, isOutput=False)
    W2_in = nc.declare_dram_parameter("W2", [F, F], bf16, isOutput=False)
    W3_in = nc.declare_dram_parameter("W3", [F, F], bf16, isOutput=False)
    ball_in = nc.declare_dram_parameter("ballw", [128, 3 * F], bf16, isOutput=False)
    iota256_in = nc.declare_dram_parameter("iota256", [128, 256], bf16, isOutput=False)
    iota64_in = nc.declare_dram_parameter("iota64", [128, F], bf16, isOutput=False)
    ident_in = nc.declare_dram_parameter("ident", [128, 128], bf16, isOutput=False)
    ones_in = nc.declare_dram_parameter("onescol", [128, 1], bf16, isOutput=False)
    dstl_in = nc.declare_dram_parameter("dstl", [128, NBLK], f32, isOutput=False)
    wsl_in = nc.declare_dram_parameter("wsl", [128, NBLK], f32, isOutput=False)
    idx_in = nc.declare_dram_parameter("idx16", [128, TOT // 16], i16, isOutput=False)
    dinv_in = nc.declare_dram_parameter("dinv", [128, W], f32, isOutput=False)
    dinvw_in = nc.declare_dram_parameter("dinvwide", [128, W * F], bf16, isOutput=False)
    bl_in = nc.declare_dram_parameter("batchloc", [128, W], f32, isOutput=False)
    pool_out = nc.declare_dram_parameter("pool_out", [F, F + 1], f32, isOutput=True)

    stk = ExitStack()
    tbl_sems = [stk.enter_context(nc.semaphore(f"tbl_{i}")) for i in range(3)]
    wrA_sems = [stk.enter_context(nc.semaphore(f"wrA_{i}")) for i in range(3)]
    wrB_sems = [stk.enter_context(nc.semaphore(f"wrB_{i}")) for i in range(3)]

    HW = W_A                   # 70 windows in A, 28 in B
    HN = W_A * 128
    GRP0_H = ([0, 14, 28, 42, 56], [0, 14])
    GRPS_H = ([14] * 5, [14] * 2)
    N_WR_H = (len(GRP0_H[0]) * C, len(GRP0_H[1]) * C)

    with tile.TileContext(nc, num_cores=C) as tc:
        tc.race_detector_enabled = False
        with (
            tc.tile_pool(name="persist", bufs=1) as pp,
            tc.tile_pool(name="idxp", bufs=3) as idxp,
            tc.tile_pool(name="msgp", bufs=3) as mp,
            tc.tile_pool(name="wstp", bufs=16) as wp,
            tc.tile_pool(name="stgp", bufs=3) as sgp,
            tc.tile_pool(name="epi", bufs=1) as ep,
            tc.tile_pool(name="ps", bufs=7, space="PSUM") as ps,
            tc.tile_pool(name="psPool", bufs=1, space="PSUM") as ps1,
            tc.tile_pool(name="dram", bufs=1, space="DRAM") as dr,
        ):
            def load(name, shape, dt, src):
                t = pp.tile(shape, dt, name=name)
                nc.sync.dma_start(out=t[:], in_=src[:])
                return t

            xT_a = load("xT_a", [IN_DIM, PADN], bf16, xT_in)
            w1 = load("w1", [IN_DIM, F], bf16, W1_in)
            w2 = load("w2", [F, F], bf16, W2_in)
            w3 = load("w3", [F, F], bf16, W3_in)
            ballw = load("ballw", [128, 3 * F], bf16, ball_in)
            iota256 = load("iota256", [128, 256], bf16, iota256_in)
            iota64 = load("iota64", [128, F], bf16, iota64_in)
            ident = load("ident", [128, 128], bf16, ident_in)
            onescol = load("onescol", [128, 1], bf16, ones_in)
            dstl = load("dstl", [128, NBLK], f32, dstl_in)
            wsl = load("wsl", [128, NBLK], f32, wsl_in)
            dinv = load("dinv", [128, W], f32, dinv_in)
            dinvw = load("dinvwide", [128, W * F], bf16, dinvw_in)
            batchloc = load("batchloc", [128, W], f32, bl_in)

            ytb = pp.tile([128, W * 128], bf16, name="ytb")
            xTn = pp.tile([F, PADN], bf16, name="xTn")
            acc = pp.tile([128, W * F], bf16, name="acc")
            xpr = pp.tile([128, W * F], bf16, name="xpr")
            rs_sbA = pp.tile([128, HW * F], bf16, name="rs_sbA")
            rs_sbB = pp.tile([128, (W - HW) * F], bf16, name="rs_sbB")

            psum_pool = ps1.tile([F, F + 1], f32, name="psum_pool")
            psum_sums = psum_pool[:, 0:F]
            psum_cnt = psum_pool[:, F:F + 1]
            pool_mm = [0]  # matmul counter for start flags

            tables = [dr.tile([PADN, 128], bf16, name=f"table_{L}")
                      for L in range(3)]
            partsA = [dr.tile([C * HN, F], bf16, name=f"partA_{L}")
                      for L in range(3)]
            partsB = [dr.tile([C * (PADN - HN), F], bf16, name=f"partB_{L}")
                      for L in range(3)]
            rsoutA = [dr.tile([HN, F], bf16, name=f"rsoutA_{L}")
                      for L in range(3)]
            rsoutB = [dr.tile([PADN - HN, F], bf16, name=f"rsoutB_{L}")
                      for L in range(3)]

            def phase_a(L, u0, u1):
                """y~ = dinv*(x@W) for windows [u0,u1); dup bf16 into ytb;
                then write the table rows for that half."""
                wmat = (w1, w2, w3)[L]
                for u in range(u0, u1):
                    psum_y = ps.tile([128, F], f32, name="psum_y", tag="psum_y",
                                     bufs=2)
                    lhsT = (xT_a if L == 0 else xTn)[:, u * 128:(u + 1) * 128]
                    nc.tensor.matmul(psum_y[:], lhsT, wmat[:], start=True,
                                     stop=True)
                    nc.vector.tensor_scalar(
                        ytb[:, u * 128:u * 128 + F], psum_y[:],
                        dinv[:, u:u + 1], None, OP.mult)
                    nc.scalar.activation(
                        ytb[:, u * 128 + F:(u + 1) * 128], psum_y[:],
                        AF.Copy, scale=dinv[:, u:u + 1])
                tbl_ap = tables[L][:].rearrange("(p u) e -> p u e", p=128)
                nc.sync.dma_start(
                    out=tbl_ap[:, u0:u1, :],
                    in_=ytb[:, u0 * 128:u1 * 128].rearrange(
                        "p (u e) -> p u e", e=128),
                ).then_inc(tbl_sems[L], 16)

            def epilogue(L, half):
                """x' = relu(dinv*(rs + y~self) + b) for one half; acc/pool."""
                u0 = 0 if half == 0 else HW
                u1 = HW if half == 0 else W
                nw = u1 - u0
                rs_sb = rs_sbA if half == 0 else rs_sbB
                rso = (rsoutA if half == 0 else rsoutB)[L]
                nc.sync.dma_start(
                    out=rs_sb[:],
                    in_=rso[:].rearrange("(p u) f -> p (u f)", p=128))
                xs = xpr[:, u0 * F:u1 * F]
                ytb_self = ytb[:].rearrange(
                    "p (u e) -> p u e", e=128)[:, u0:u1, 0:F]
                nc.vector.tensor_tensor(xs, rs_sb[:], ytb_self, OP.add)
                nc.vector.tensor_tensor(xs, xs, dinvw[:, u0 * F:u1 * F],
                                        OP.mult)
                if has_bias:
                    for u in range(u0, u1):
                        nc.vector.tensor_tensor(
                            xpr[:, u * F:(u + 1) * F],
                            xpr[:, u * F:(u + 1) * F],
                            ballw[:, L * F:(L + 1) * F], OP.add)
                nc.vector.tensor_scalar(xs, xs, 0.0, None, OP.max)
                if L == 0:
                    nc.vector.tensor_copy(acc[:, u0 * F:u1 * F], xs)
                elif L == 1:
                    nc.vector.tensor_tensor(acc[:, u0 * F:u1 * F],
                                            acc[:, u0 * F:u1 * F], xs, OP.add)
                if L < 2:
                    for u in range(u0, u1):
                        ptr = ps.tile([F, 128], bf16, name="ptr", tag="ptr",
                                      bufs=2)
                        nc.tensor.transpose(ptr[:], xpr[:, u * F:(u + 1) * F],
                                            ident[:])
                        if u % 3 == 2:
                            nc.scalar.activation(
                                xTn[:, u * 128:(u + 1) * 128], ptr[:], AF.Copy)
                        else:
                            nc.vector.tensor_copy(
                                xTn[:, u * 128:(u + 1) * 128], ptr[:])
                    phase_a(L + 1, u0, u1)
                else:
                    pool_pass(xpr, u0, u1, last=(half == 1))

            def pool_pass(src_tile, u0, u1, last):
                for u in range(u0, u1):
                    sg = wp.tile([128, 256], bf16, name="sg", tag="wst")
                    nc.vector.tensor_scalar(
                        sg[:, :F], iota64[:], batchloc[:, u:u + 1], None,
                        OP.is_equal)
                    first = pool_mm[0] == 0
                    stop = last and (u == u1 - 1)
                    nc.tensor.matmul(
                        psum_sums, sg[:, :F], src_tile[:, u * F:(u + 1) * F],
                        start=first, stop=stop, skip_group_check=True)
                    nc.tensor.matmul(
                        psum_cnt, sg[:, :F], onescol[:],
                        start=first, stop=stop, skip_group_check=True)
                    pool_mm[0] += 1

            # ---------------- initial phase A (layer 0) ---------------------
            phase_a(0, 0, HW)
            phase_a(0, HW, W)

            for L in range(3):
                partial_h = (partsA[L], partsB[L])
                rsout_h = (rsoutA[L], rsoutB[L])
                wr_h = (wrA_sems[L], wrB_sems[L])

                nc.gpsimd.wait_ge(tbl_sems[L], 32)

                # pooling pass 1 over acc = x1+x2 runs during layer-2 blocks
                if L == 2:
                    pool_pass(acc, 0, W, last=False)

                n_ops = len(op_sizes)
                mts = [None] * n_ops
                op_base_blk = [0] * n_ops
                off = 0
                for k, s in enumerate(op_sizes):
                    op_base_blk[k] = off // 128
                    off += s

                size_regs = {}
                for s in set(op_sizes):
                    size_regs[s] = nc.gpsimd.to_reg(s)

                def issue_gather(k):
                    s = op_sizes[k]
                    off16 = sum(op_sizes[:k]) // 16
                    it = idxp.tile([128, OPN // 16], i16, name="idxt",
                                   tag="idxt")
                    nc.gpsimd.dma_start(out=it[:, :s // 16],
                                        in_=idx_in[:, off16:off16 + s // 16])
                    mt = mp.tile([128, OPN // 128, 128], bf16, name="msg",
                                 tag="msg")
                    nc.gpsimd.dma_gather(
                        out_ap=mt[:, :s // 128, :],
                        in_ap=tables[L][:],
                        idxs_ap=it[:, :s // 16],
                        num_idxs=s,
                        num_idxs_reg=size_regs[s],
                        elem_size=128,
                    )
                    mts[k] = mt

                issue_gather(0)
                if n_ops > 1:
                    issue_gather(1)

                psums = {}
                stg = {}

                def window_close(wn):
                    u = wn % W
                    kk = wn // W
                    half = 0 if u < HW else 1
                    uu = u - (0 if half == 0 else HW)
                    grp = min(uu // 14, len(GRP0_H[half]) - 1)
                    gkey = (kk, half, grp)
                    sz = GRPS_H[half][grp]
                    if gkey not in stg:
                        stg[gkey] = sgp.tile([128, 14 * F], bf16, name="stg",
                                             tag="stg")
                    voff = uu - GRP0_H[half][grp]
                    nc.scalar.activation(
                        stg[gkey][:, voff * F:(voff + 1) * F],
                        psums.pop(wn)[:], AF.Copy)
                    if voff == sz - 1:
                        part = partial_h[half]
                        nh = HW if half == 0 else W - HW
                        g0 = GRP0_H[half][grp]
                        dst_ap = part[:].rearrange(
                            "(k p u) f -> k p (u f)", k=C, p=128
                        )[kk][:, g0 * F:(g0 + sz) * F]
                        nc.sync.dma_start(
                            out=dst_ap, in_=stg.pop(gkey)[:, :sz * F]
                        ).then_inc(wr_h[half], 16)

                nA_blk = prep["nA_blk"]
                trigger_blk = nA_blk + int(0.55 * (NBLK - nA_blk))
                closedA = [0]

                m = 0
                cur_op = 0
                for blk in range(NBLK):
                    while cur_op + 1 < n_ops and blk >= op_base_blk[cur_op + 1]:
                        cur_op += 1
                    for ahead in (1, 2):
                        if cur_op + ahead < n_ops and mts[cur_op + ahead] is None:
                            issue_gather(cur_op + ahead)
                    mt = mts[cur_op]
                    jloc = blk - op_base_blk[cur_op]

                    sec = bool(prep["sec_needed"][blk])
                    width = 256 if sec else 128
                    wst = wp.tile([128, 256], bf16, name="wst", tag="wst")
                    nc.vector.tensor_scalar(
                        wst[:, :width], iota256[:, :width],
                        dstl[:, blk:blk + 1], wsl[:, blk:blk + 1],
                        OP.is_equal, OP.mult)

                    for half_mm in range(2 if sec else 1):
                        wn = int(mm_win[m])
                        if wn not in psums:
                            psums[wn] = ps.tile([128, F], f32, name="psum_w",
                                                tag="psum_w", bufs=3)
                        nc.tensor.matmul(
                            psums[wn][:],
                            wst[:, half_mm * 128:(half_mm + 1) * 128],
                            mt[:, jloc, 0:F],
                            start=(m == first_mm[wn]), stop=(m == last_mm[wn]),
                            skip_group_check=True)
                        m += 1
                        if (m - 1) == last_mm[wn]:
                            window_close(wn)
                            if (wn % W) < HW:
                                closedA[0] += 1
                                if closedA[0] == HW * C:
                                    # all A windows staged: kick RS_A
                                    nc.gpsimd.wait_ge(wr_h[0], 16 * N_WR_H[0])
                                    nc.gpsimd.collective_compute(
                                        "ReduceScatter", OP.add,
                                        replica_groups=[list(range(C))],
                                        ins=[partial_h[0][:]],
                                        outs=[rsout_h[0][:]],
                                    )

                    if blk == trigger_blk:
                        epilogue(L, 0)

                assert m == NMM

                # RS_B + exposed B boundary
                nc.gpsimd.wait_ge(wr_h[1], 16 * N_WR_H[1])
                nc.gpsimd.collective_compute(
                    "ReduceScatter", OP.add,
                    replica_groups=[list(range(C))],
                    ins=[partial_h[1][:]],
                    outs=[rsout_h[1][:]],
                )
                epilogue(L, 1)

            # ---------------- pooling output ---------------------------------
            outt = ep.tile([F, F + 1], f32, name="outt")
            nc.vector.tensor_copy(outt[:, :F], psum_sums)
            nc.vector.tensor_copy(outt[:, F:F + 1], psum_cnt)
            nc.sync.dma_start(out=pool_out[:], in_=outt[:])

    stk.close()
    nc.compile()
    _split_waits(nc)
    return nc


# --------------------------------------------------------------------------
def kernel(x, edge_weight, W1, b1, W2, b2, W3, b3, Wl, bl, edge_index, batch):
    from concourse.bass_utils import run_bass_kernel_spmd
    import jax.numpy as jnp

    prep = _host_prep(x, edge_weight, edge_index, batch)

    has_bias = any(np.any(np.asarray(b) != 0) for b in (b1, b2, b3))
    cache_key = (prep["NBLK"], prep["NMM"], tuple(prep["op_sizes"][:3]), has_bias)
    if cache_key not in _prog_cache:
        _prog_cache[cache_key] = _build_program(prep, has_bias=has_bias)
    nc = _prog_cache[cache_key]

    bf = lambda a: np.asarray(jnp.asarray(np.asarray(a, np.float32), jnp.bfloat16))
    W1b, W2b, W3b = bf(W1), bf(W2), bf(W3)
    ballw = np.zeros((128, 3 * F), dtype=np.float32)
    ballw[:, 0:F] = np.asarray(b1, np.float32)[None, :]
    ballw[:, F:2 * F] = np.asarray(b2, np.float32)[None, :]
    ballw[:, 2 * F:3 * F] = np.asarray(b3, np.float32)[None, :]
    ballw = bf(ballw)
    iota256 = bf(np.tile(np.arange(256, dtype=np.float32)[None, :], (128, 1)))
    iota64 = bf(np.tile(np.arange(F, dtype=np.float32)[None, :], (128, 1)))
    ident = bf(np.eye(128, dtype=np.float32))
    onescol = bf(np.ones((128, 1), dtype=np.float32))
    xT_bf = bf(prep["xT"])
    dinvw = bf(np.repeat(prep["dinv_lane"], F, axis=2))  # [C,128,W*F]

    in_maps = []
    for c in range(C):
        in_maps.append({
            "xT": xT_bf[c],
            "W1": W1b, "W2": W2b, "W3": W3b, "ballw": ballw,
            "iota256": iota256, "iota64": iota64, "ident": ident,
            "onescol": onescol,
            "dstl": prep["dstl_blk"][c], "wsl": prep["wsl_blk"][c],
            "idx16": prep["idx_full"][c],
            "dinv": prep["dinv_lane"][c], "dinvwide": dinvw[c],
            "batchloc": prep["bl_lane"][c],
        })

    res = run_bass_kernel_spmd(nc, in_maps, core_ids=list(range(C)))

    sums = np.zeros((N_GRAPHS, F), dtype=np.float64)
    cnts = np.zeros(N_GRAPHS, dtype=np.float64)
    for c in range(C):
        out = res.results[c]["pool_out"]
        g0 = int(prep["gmin"][c])
        for r in range(63):
            g = g0 + r
            if g < N_GRAPHS:
                sums[g] += out[r, :F]
                cnts[g] += out[r, F]
    pooled = (sums / 3.0) / np.maximum(cnts, 1.0)[:, None]
    logits = pooled @ np.asarray(Wl, np.float64) + np.asarray(bl, np.float64)
    z = logits - logits.max(axis=1, keepdims=True)
    ez = np.exp(z)
    return (ez / ez.sum(axis=1, keepdims=True)).astype(np.float32)
